# revision 1
# baseline (speedup 1.0000x reference)
"""Multi-head attention (B=2, S=2048, D=1024, H=16) on 8 Trainium2 NeuronCores.

Sharding: core c handles (batch b=c//4, query chunk j=c%4 of 512 rows).
 - Each core computes K^T / V for its WHOLE batch locally (weights replicated,
   pre-transposed + bf16-cast on host; softmax scale folded into W_q) — no
   collectives, the PE stays continuously busy so the HAM clock-gate stays
   warm.
 - Q projected for the core's own 512 rows only.
 - Attention (all 16 heads, 512 queries x 2048 keys):
   scoresT = K_h @ Q_h^T  ->  exp on ACT  ->  attnT = [V_h|1]^T @ E
   (ones column gives the softmax denominator Z in row 64 of attnT psum).
 - Q/K biases folded into the projection casts (ACT Identity per-partition
   bias); V bias folded into the output bias on host (sum(probs) == 1).
 - Output projection local per 512-row chunk; final output assembled on host.
"""

import numpy as np
import ml_dtypes

import concourse.bass as bass
import concourse.mybir as mybir
import concourse.tile as tile
from concourse import bacc
from concourse.bass_utils import run_bass_kernel_spmd

BF16 = mybir.dt.bfloat16
F32 = mybir.dt.float32
AF = mybir.ActivationFunctionType

B, S, D = 2, 2048, 1024
H, HD = 16, 64
N_CORES = 8
R = 4            # cores per batch
SL = S // R      # local query rows per core (512)
P = 128
DCH = D // P     # 8 d-chunks
NKK = S // P     # 16 key chunks
ET = D // P      # 8 feature tiles per projection
FREE = 512


def build_program():
    nc = bacc.Bacc("TRN2", target_bir_lowering=False, debug=False,
                   num_devices=N_CORES)

    xT = nc.dram_tensor("xT", [D, S], BF16, kind="ExternalInput")
    xqT = nc.dram_tensor("xqT", [D, SL], BF16, kind="ExternalInput")
    wqkvT = nc.dram_tensor("wqkvT", [D, 3 * D], BF16, kind="ExternalInput")
    bqk = nc.dram_tensor("bqk", [P, 16], BF16, kind="ExternalInput")
    woutT = nc.dram_tensor("woutT", [D, D], BF16, kind="ExternalInput")
    bout = nc.dram_tensor("bout", [1, D], BF16, kind="ExternalInput")
    out = nc.dram_tensor("out", [SL, D], F32, kind="ExternalOutput")

    with tile.TileContext(nc) as tc:
        _build(nc, tc, xT, xqT, wqkvT, bqk, woutT, bout, out)
    nc.compile()
    return nc


def _build(nc, tc, xT, xqT, wqkvT, bqk, woutT, bout, out):
    from contextlib import ExitStack

    ctx = ExitStack()
    consts = ctx.enter_context(tc.tile_pool(name="consts", bufs=1))

    # ---- constants ----
    ones_bf = consts.tile([1, FREE], BF16, name="ones_bf")
    nc.vector.memset(ones_bf[:], 1.0)
    bqk_sb = consts.tile([P, 16], BF16, name="bqk_sb")
    nc.sync.dma_start(bqk_sb[:], bqk.ap())
    bout_sb = consts.tile([1, D], BF16, name="bout_sb")
    nc.sync.dma_start(bout_sb[:], bout.ap())

    # ---- resident input tiles ----
    xt_pool = ctx.enter_context(tc.tile_pool(name="xt", bufs=1))
    xt = []
    for i in range(DCH):
        t = xt_pool.tile([P, S], BF16, name=f"xt{i}")
        for ch in range(4):
            nc.sync.dma_start(t[:, FREE * ch:FREE * (ch + 1)],
                              xT.ap()[P * i:P * (i + 1),
                                      FREE * ch:FREE * (ch + 1)])
        xt.append(t)
    xq = []
    for i in range(DCH):
        t = xt_pool.tile([P, SL], BF16, name=f"xq{i}")
        nc.sync.dma_start(t[:], xqT.ap()[P * i:P * (i + 1), :])
        xq.append(t)

    # ---- weight stream (K, V blocks first, then Q) ----
    w_pool = ctx.enter_context(tc.tile_pool(name="wq", bufs=24))
    wblk = {}

    def load_w(ebs):
        for eb in ebs:
            for d in range(DCH):
                t = w_pool.tile([P, FREE], BF16, name=f"w{eb}_{d}", tag="w")
                nc.gpsimd.dma_start(t[:], wqkvT.ap()[P * d:P * (d + 1),
                                                     FREE * eb:FREE * (eb + 1)])
                wblk[(eb, d)] = t

    # V layout: per key-tile, 16 heads x (64 V-features + ones col) packed at
    # stride 65, plus 64 zero columns of tail pad so a 128-wide stationary
    # slice [65h : 65h+128] is always in bounds. Columns 65..127 of that
    # slice hit neighbor-head data; they only feed psum rows 65..127 which
    # are never read. Full 128x128 stationary keeps the PE clock-gate warm.
    VW = H * (HD + 1) + HD  # 1104
    kv_pool = ctx.enter_context(tc.tile_pool(name="kv", bufs=1))
    kt = [kv_pool.tile([P, S], BF16, name=f"kt{t}") for t in range(ET)]
    vt = [kv_pool.tile([P, VW], BF16, name=f"vt{g}") for g in range(NKK)]
    for g in range(NKK):
        v3 = vt[g][:, 0:H * (HD + 1)].rearrange("p (h c) -> p h c", c=HD + 1)
        nc.vector.memset(v3[:, :, HD:HD + 1], 1.0)
        nc.vector.memset(vt[g][:, H * (HD + 1):VW], 0.0)
    # Q, zero-padded per head: head h occupies partitions 64*(h%2)..+64 of
    # qz[h], the other 64 partitions are zero -> scores matmul can use the
    # full 128-partition K^T pair tile as stationary (K=128, stays warm).
    qt_pool = ctx.enter_context(tc.tile_pool(name="qt", bufs=1))
    qz = [qt_pool.tile([P, FREE], BF16, name=f"qz{h}") for h in range(H)]
    for h in range(H):
        off = HD * ((h + 1) % 2)
        nc.vector.memset(qz[h][off:off + HD, :], 0.0)

    # ---- K^T projection, full batch: out[e, s] ----
    load_w((2, 3))
    with tc.tile_pool(name="projk_ps", bufs=8, space="PSUM") as ps_pool:
        for t in range(ET):
            eb = 2 + t // 4
            co = P * (t % 4)
            pss = [ps_pool.tile([P, FREE], F32, name=f"psk{t}_{sch}",
                                tag="proj") for sch in range(4)]
            # d-loop outer: 4 consecutive matmuls share one stationary tile
            for d in range(DCH):
                for sch in range(4):
                    nc.tensor.matmul(pss[sch][:],
                                     wblk[(eb, d)][:, co:co + P],
                                     xt[d][:, FREE * sch:FREE * (sch + 1)],
                                     start=(d == 0), stop=(d == DCH - 1))
            for sch in range(4):
                # cast + K-bias (per-partition) fused on ACT
                nc.scalar.activation(kt[t][:, FREE * sch:FREE * (sch + 1)],
                                     pss[sch][:], AF.Identity,
                                     bias=bqk_sb[:, 8 + t:9 + t])

    # ---- V projection, full batch, natural: out[s, e] ----
    load_w((4, 5))
    load_w((0, 1))
    with tc.tile_pool(name="projv_ps", bufs=8, space="PSUM") as ps_pool:
        for st in range(NKK):
            pss = [ps_pool.tile([P, FREE], F32, name=f"psv{st}_{eb}",
                                tag="proj") for eb in range(2)]
            for d in range(DCH):
                for eb in range(2):
                    nc.tensor.matmul(pss[eb][:],
                                     xt[d][:, P * st:P * (st + 1)],
                                     wblk[(4 + eb, d)][:],
                                     start=(d == 0), stop=(d == DCH - 1))
            for eb in range(2):
                # cast to the [V|1] attention layout on DVE (no bias:
                # V-bias is folded into the output bias on host)
                v3 = vt[st][:, 0:H * (HD + 1)].rearrange(
                    "p (h c) -> p h c", c=HD + 1)
                nc.vector.tensor_copy(
                    v3[:, 8 * eb:8 * (eb + 1), 0:HD],
                    pss[eb].rearrange("p (h d) -> p h d", d=HD))

    # ---- Q projection (own 512 rows): out[e, q] ----
    with tc.tile_pool(name="projq_ps", bufs=4, space="PSUM") as ps_pool:
        for t in range(ET):
            eb = t // 4
            co = P * (t % 4)
            ps = ps_pool.tile([P, FREE], F32, name=f"psq{t}", tag="proj")
            for d in range(DCH):
                nc.tensor.matmul(ps[:], wblk[(eb, d)][:, co:co + P], xq[d][:],
                                 start=(d == 0), stop=(d == DCH - 1))
            nc.scalar.activation(qz[2 * t][0:HD, :], ps[0:HD, :], AF.Identity,
                                 bias=bqk_sb[0:HD, t:t + 1])
            nc.scalar.activation(qz[2 * t + 1][HD:P, :], ps[HD:P, :],
                                 AF.Identity, bias=bqk_sb[HD:P, t:t + 1])

    # ---- prefetch output-projection weights ----
    wo_pool = ctx.enter_context(tc.tile_pool(name="wo", bufs=1))
    wo = []
    for p_ in range(DCH):
        t = wo_pool.tile([P, D], BF16, name=f"wo{p_}")
        nc.sync.dma_start(t[:], woutT.ap()[P * p_:P * (p_ + 1), :])
        wo.append(t)

    # ---- attention, with the output projection interleaved ----
    # outproj partials accumulate into SBUF f32 via DVE, so only one extra
    # PSUM bank is needed and the outproj hides inside attention's ACT slack.
    attn_sb_pool = ctx.enter_context(tc.tile_pool(name="attnsb", bufs=1))
    small_pool = ctx.enter_context(tc.tile_pool(name="small", bufs=2))
    osb_pool = ctx.enter_context(tc.tile_pool(name="osb", bufs=1))
    attn_sb = [attn_sb_pool.tile([P, FREE], BF16, name=f"attnsb{p_}")
               for p_ in range(H // 2)]
    osb = [osb_pool.tile([P, D], F32, name=f"osb{st}")
           for st in range(SL // P)]
    GRP = 2  # kk-chunks per score-psum tile
    with tc.tile_pool(name="sc_ps", bufs=2, space="PSUM") as sc_ps, \
         tc.tile_pool(name="atbc_ps", bufs=1, space="PSUM") as atbc_ps, \
         tc.tile_pool(name="op_ps", bufs=3, space="PSUM") as op_ps, \
         tc.tile_pool(name="e_sb", bufs=3) as e_pool:
        # init osb with the output bias (broadcast via PE outer-product)
        for st in range(SL // P):
            for eb in range(2):
                bi = op_ps.tile([P, FREE], F32, name=f"bi{st}_{eb}", tag="op")
                nc.tensor.matmul(bi[:], ones_bf[:, :P],
                                 bout_sb[:, FREE * eb:FREE * (eb + 1)],
                                 start=True, stop=True)
                nc.vector.tensor_copy(osb[st][:, FREE * eb:FREE * (eb + 1)],
                                      bi[:])

        pv_pending = []
        norm_steps = []
        op_tasks = []
        op_stage = []
        op_stage2 = []

        def normalize_steps(h, at):
            # spread the normalize chain across groups so no single DVE
            # instruction (esp. the reciprocal) dams the DVE queue that the
            # interleaved outproj adds also run on
            koff = HD * (h % 2)
            # atsb is created by attn_v's final group, which pops one group
            # after these steps are queued -> look it up lazily at call time
            rz = small_pool.tile([1, FREE], F32, name=f"rz{h}", tag="rz")
            rzb = small_pool.tile([HD, FREE], F32, name=f"rzb{h}", tag="rzb")

            def s_recip(i):
                def f():
                    atsb = atsb_map[h]
                    nc.vector.reciprocal(rz[0:1, P * i:P * (i + 1)],
                                         atsb[HD:HD + 1, P * i:P * (i + 1)])
                return f

            def s_pb():
                nc.gpsimd.partition_broadcast(rzb[:], rz[:])

            def s_mul():
                atsb = atsb_map.pop(h)
                nc.vector.tensor_mul(attn_sb[h // 2][koff:koff + HD, :],
                                     atsb[0:HD, :], rzb[:])
                if h % 2 == 1:
                    p_ = h // 2
                    op_tasks.extend(op_stage)
                    op_stage.clear()
                    op_stage.extend(op_stage2)
                    op_stage2.clear()
                    for st in range(SL // P):
                        for eb in range(2):
                            op_stage2.append((p_, st, eb))

            return [s_recip(0), s_recip(1), s_recip(2), s_recip(3),
                    s_pb, s_mul]

        def run_op(p_, st, eb):
            op = op_ps.tile([P, FREE], F32, name=f"op{p_}_{st}_{eb}",
                            tag="op")
            nc.tensor.matmul(op[:], attn_sb[p_][:, P * st:P * (st + 1)],
                             wo[p_][:, FREE * eb:FREE * (eb + 1)],
                             start=True, stop=True)
            nc.vector.tensor_add(osb[st][:, FREE * eb:FREE * (eb + 1)],
                                 osb[st][:, FREE * eb:FREE * (eb + 1)],
                                 op[:])

        atsb_map = {}

        def attn_v(h, at, g, e):
            for j in range(GRP):
                kk = GRP * g + j
                nc.tensor.matmul(at[:], vt[kk][:, 65 * h:65 * h + P],
                                 e[:, FREE * j:FREE * (j + 1)],
                                 start=(kk == 0), stop=(kk == NKK - 1))
            if g == NKK // GRP - 1:
                atsb = small_pool.tile([HD + 1, FREE], F32, name=f"atsb{h}",
                                       tag="atsb")
                nc.vector.tensor_copy(atsb[:], at[0:HD + 1, :])
                atsb_map[h] = atsb

        for h in range(H):
            ktile = h // 2
            q_rhs = qz[h][:]
            at = atbc_ps.tile([P, FREE], F32, name=f"at{h}", tag="atbc")
            for g in range(NKK // GRP):
                sc = sc_ps.tile([P, GRP * FREE], F32, name=f"sc{h}_{g}",
                                tag="sc")
                for j in range(GRP):
                    kk = GRP * g + j
                    nc.tensor.matmul(
                        sc[:, FREE * j:FREE * (j + 1)],
                        kt[ktile][:, P * kk:P * (kk + 1)],
                        q_rhs, start=True, stop=True)
                e = e_pool.tile([P, GRP * FREE], BF16, name=f"e{h}_{g}",
                                tag="e")
                nc.scalar.activation(e[:], sc[:], AF.Exp)
                # run the PV matmuls one group behind the scores stream, so
                # the PE never waits on the exp it just produced
                if pv_pending:
                    attn_v(*pv_pending.pop())
                pv_pending.append((h, at, g, e))
                if norm_steps:
                    norm_steps.pop(0)()
                # drain outproj at the task arrival rate (8 per 2 heads) so
                # the DVE adds never saturate a single head's budget; drain
                # at full rate near the end so the post-loop flush only has
                # the final pair left
                if (g % 2 == 1 or h >= 11) and op_tasks:
                    run_op(*op_tasks.pop(0))
            norm_steps.extend(normalize_steps(h, at))
        attn_v(*pv_pending.pop())
        for f in norm_steps:
            f()
        op_tasks.extend(op_stage)
        op_tasks.extend(op_stage2)
        while op_tasks:
            run_op(*op_tasks.pop(0))
        for st in range(SL // P):
            nc.sync.dma_start(out.ap()[P * st:P * (st + 1), :], osb[st][:])

    ctx.close()


_CACHE = {}


def _get_program():
    if "nc" not in _CACHE:
        _CACHE["nc"] = build_program()
    return _CACHE["nc"]


def prep_inputs(input_tensor, qkv_weight, qkv_bias, out_weight, out_bias):
    """Host-side shard + transpose + cast. Returns in_maps for 8 cores."""
    x = np.asarray(input_tensor, np.float32)
    wqkv = np.asarray(qkv_weight, np.float32).copy()
    bq = np.asarray(qkv_bias, np.float32).copy()
    wout = np.asarray(out_weight, np.float32)
    scale = 1.0 / np.sqrt(np.float32(HD))
    wqkv[:D] *= scale
    bq[:D] *= scale
    bf = ml_dtypes.bfloat16
    wqkvT = np.ascontiguousarray(wqkv.T).astype(bf)
    # Q/K biases, column-major per 128-feature tile: bqk[:, t] = bias tile t
    bqk = np.ascontiguousarray(bq[:2 * D].reshape(16, P).T).astype(bf)
    woutT = np.ascontiguousarray(wout.T).astype(bf)
    # V bias folded into output bias: probs @ (V + b_v) = probs @ V + b_v
    bout_eff = np.asarray(out_bias, np.float32) + wout @ bq[2 * D:]
    bout = bout_eff.reshape(1, D).astype(bf)
    xTb = [np.ascontiguousarray(x[b].T).astype(bf) for b in range(B)]
    in_maps = []
    for c in range(N_CORES):
        b, j = c // R, c % R
        xqT = np.ascontiguousarray(xTb[b][:, SL * j:SL * (j + 1)])
        in_maps.append({"xT": xTb[b], "xqT": xqT, "wqkvT": wqkvT,
                        "bqk": bqk, "woutT": woutT, "bout": bout})
    return in_maps


def kernel(input_tensor, qkv_weight, qkv_bias, out_weight, out_bias,
           **run_kwargs):
    nc = _get_program()
    in_maps = prep_inputs(input_tensor, qkv_weight, qkv_bias, out_weight,
                          out_bias)
    res = run_bass_kernel_spmd(nc, in_maps, core_ids=list(range(N_CORES)),
                               **run_kwargs)
    full = np.empty((B, S, D), np.float32)
    for c in range(N_CORES):
        b, j = c // R, c % R
        full[b, SL * j:SL * (j + 1), :] = res.results[c]["out"]
    if run_kwargs:
        kernel.last_results = res
    return full



# revision 11
# speedup vs baseline: 1.6336x; 1.6336x over previous
"""Multi-head attention (B=2, S=2048, D=1024, H=16) on 8 Trainium2 NeuronCores.

Sharding: core c handles (batch b=c//4, head-group g=c%4 of 4 heads) for ALL
2048 queries — head/tensor parallel instead of the old query-parallel split.
 - Q/K/V projections only cover the core's 256 features (4x less PE work than
   replicating K/V per batch; no collectives needed).
 - Attention (4 heads x 2048 queries x 2048 keys):
   scores^T = K_h^T-pair @ Q_h^T as K=64-contraction matmuls in alternating
   PE row groups (two heads run concurrently in the array),
   exp on ACT at FD=1024, attnT = [V_h|1]^T @ E with 65-col stationaries
   (ones column gives the softmax denominator Z in psum row 64).
 - Normalize uses the fast approximate reciprocal custom DVE op.
 - Output projection contracts only the local 256 features -> each core emits
   a PARTIAL output [2048, 1024] bf16; the host sums the 4 partials per batch
   and adds the (V-bias-folded) output bias.
"""

import numpy as np
import ml_dtypes

import concourse.bass as bass
import concourse.mybir as mybir
import concourse.tile as tile
from concourse import bacc
from concourse.bass_utils import run_bass_kernel_spmd

BF16 = mybir.dt.bfloat16
F32 = mybir.dt.float32
AF = mybir.ActivationFunctionType

B, S, D = 2, 2048, 1024
H, HD = 16, 64
N_CORES = 8
G = 4              # head-groups per batch (cores per batch)
HL = H // G        # heads per core (4)
FL = HL * HD       # local projected features (256)
P = 128
DCH = D // P       # 8 contraction chunks
NKK = S // P       # 16 key chunks
QC = 512           # query block
NQC = S // QC      # 4
VW = HL * (HD + 1) + HD  # packed [V|1] width + 64 pad so 65h+65 slices stay
                         # inside one dense region (pad cols memset to 0)


DEBUG_DUMP = False


def build_program():
    nc = bacc.Bacc("TRN2", target_bir_lowering=False, debug=False,
                   num_devices=N_CORES)

    xT = nc.dram_tensor("xT", [D, S], BF16, kind="ExternalInput")
    wqT = nc.dram_tensor("wqT", [D, FL], BF16, kind="ExternalInput")
    wkT = nc.dram_tensor("wkT", [D, FL], BF16, kind="ExternalInput")
    wvT = nc.dram_tensor("wvT", [D, FL], BF16, kind="ExternalInput")
    woT = nc.dram_tensor("woT", [FL, D], BF16, kind="ExternalInput")
    bqk = nc.dram_tensor("bqk", [P, 4], BF16, kind="ExternalInput")
    out = nc.dram_tensor("out", [S, D], BF16, kind="ExternalOutput")
    dbg = {}
    if DEBUG_DUMP:
        for nm, shape, dt in (
                ("dbg_kt0", [P, S], BF16), ("dbg_qp0", [P, S], BF16),
                ("dbg_vt0", [P, VW], BF16), ("dbg_e", [P, 2 * QC], BF16),
                ("dbg_atsb", [HD + 1, QC], F32), ("dbg_rz", [1, QC], F32),
                ("dbg_rzb", [HD, QC], F32), ("dbg_asb0", [P, S], BF16)):
            dbg[nm] = nc.dram_tensor(nm, shape, dt, kind="ExternalOutput")

    with tile.TileContext(nc) as tc:
        _build(nc, tc, xT, wqT, wkT, wvT, woT, bqk, out, dbg)
    nc.compile()
    return nc


def _build(nc, tc, xT, wqT, wkT, wvT, woT, bqk, out, dbg=()):
    from contextlib import ExitStack

    ctx = ExitStack()
    consts = ctx.enter_context(tc.tile_pool(name="consts", bufs=1))
    bqk_sb = consts.tile([P, 4], BF16, name="bqk_sb")
    nc.sync.dma_start(bqk_sb[:], bqk.ap())

    # ---- resident input tiles ----
    xt_pool = ctx.enter_context(tc.tile_pool(name="xt", bufs=1))
    xt = []
    for i in range(DCH):
        t = xt_pool.tile([P, S], BF16, name=f"xt{i}")
        for ch in range(4):
            nc.sync.dma_start(t[:, QC * ch:QC * (ch + 1)],
                              xT.ap()[P * i:P * (i + 1),
                                      QC * ch:QC * (ch + 1)])
        xt.append(t)

    # ---- weights ----
    w_pool = ctx.enter_context(tc.tile_pool(name="w", bufs=1))
    wk, wq, wv = [], [], []
    for nm, dram, lst in (("wk", wkT, wk), ("wq", wqT, wq), ("wv", wvT, wv)):
        for d in range(DCH):
            t = w_pool.tile([P, FL], BF16, name=f"{nm}{d}")
            nc.gpsimd.dma_start(t[:], dram.ap()[P * d:P * (d + 1), :])
            lst.append(t)
    wo = []
    for p_ in range(2):
        t = w_pool.tile([P, D], BF16, name=f"wo{p_}")
        nc.gpsimd.dma_start(t[:], woT.ap()[P * p_:P * (p_ + 1), :])
        wo.append(t)

    # ---- persistent compute tiles ----
    kv_pool = ctx.enter_context(tc.tile_pool(name="kv", bufs=1))
    kt = [kv_pool.tile([P, S], BF16, name=f"kt{t}") for t in range(2)]
    qp = [kv_pool.tile([P, S], BF16, name=f"qp{t}") for t in range(2)]
    vt = [kv_pool.tile([P, VW], BF16, name=f"vt{g}") for g in range(NKK)]
    for g in range(NKK):
        v3 = vt[g][:, 0:HL * (HD + 1)].rearrange("p (h c) -> p h c", c=HD + 1)
        nc.vector.memset(v3[:, :, HD:HD + 1], 1.0)
        nc.vector.memset(vt[g][:, HL * (HD + 1):VW], 0.0)
    attn_sb = [kv_pool.tile([P, S], BF16, name=f"asb{t}") for t in range(2)]
    osb = [kv_pool.tile([P, D], BF16, name=f"osb{st}")
           for st in range(S // P)]

    # ---- K projection: kt[t][f, s], f-pair tile t ----
    with tc.tile_pool(name="projkq_ps", bufs=2, space="PSUM") as ps_pool:
        for t in range(2):
            ps = ps_pool.tile([P, S], F32, name=f"psk{t}", tag="proj")
            for d in range(DCH):
                for sch in range(4):
                    nc.tensor.matmul(ps[:, QC * sch:QC * (sch + 1)],
                                     wk[d][:, P * t:P * (t + 1)],
                                     xt[d][:, QC * sch:QC * (sch + 1)],
                                     start=(d == 0), stop=(d == DCH - 1))
            nc.scalar.activation(kt[t][:], ps[:], AF.Identity,
                                 bias=bqk_sb[:, 2 + t:3 + t])
            if dbg and t == 0:
                nc.sync.dma_start(dbg["dbg_kt0"].ap(), kt[0][:])
        # ---- Q projection: qp[t][f, q] ----
        for t in range(2):
            ps = ps_pool.tile([P, S], F32, name=f"psq{t}", tag="proj")
            for d in range(DCH):
                for sch in range(4):
                    nc.tensor.matmul(ps[:, QC * sch:QC * (sch + 1)],
                                     wq[d][:, P * t:P * (t + 1)],
                                     xt[d][:, QC * sch:QC * (sch + 1)],
                                     start=(d == 0), stop=(d == DCH - 1))
            nc.scalar.activation(qp[t][0:HD, :], ps[0:HD, :], AF.Identity,
                                 bias=bqk_sb[0:HD, t:t + 1])
            nc.scalar.activation(qp[t][HD:P, :], ps[HD:P, :], AF.Identity,
                                 bias=bqk_sb[HD:P, t:t + 1])
            if dbg and t == 0:
                nc.sync.dma_start(dbg["dbg_qp0"].ap(), qp[0][:])

    # ---- V projection: natural V[s, e], packed [V|1] layout ----
    with tc.tile_pool(name="projv_ps", bufs=3, space="PSUM") as ps_pool:
        for pr in range(NKK // 2):
            ps = ps_pool.tile([P, 2 * FL], F32, name=f"psv{pr}", tag="proj")
            for half in range(2):
                st = 2 * pr + half
                for d in range(DCH):
                    nc.tensor.matmul(ps[:, FL * half:FL * (half + 1)],
                                     xt[d][:, P * st:P * (st + 1)],
                                     wv[d][:],
                                     start=(d == 0), stop=(d == DCH - 1))
            for half in range(2):
                st = 2 * pr + half
                v3 = vt[st][:, 0:HL * (HD + 1)].rearrange(
                    "p (h c) -> p h c", c=HD + 1)
                nc.vector.tensor_copy(
                    v3[:, :, 0:HD],
                    ps[:, FL * half:FL * (half + 1)].rearrange(
                        "p (h dd) -> p h dd", dd=HD))
                if dbg and st == 0:
                    nc.sync.dma_start(dbg["dbg_vt0"].ap(), vt[0][:])

    # ---- attention + interleaved output projection ----
    small_pool = ctx.enter_context(tc.tile_pool(name="small", bufs=4))
    op_tasks = []

    def run_op(st):
        # partial out[s-chunk, :] = sum_p attn_sb[p][:, chunk]^T @ wo[p]
        ops = [op_ps.tile([P, QC], F32, name=f"op{st}_{eb}", tag="op")
               for eb in range(2)]
        for p_ in range(2):
            for eb in range(2):
                nc.tensor.matmul(ops[eb][:],
                                 attn_sb[p_][:, P * st:P * (st + 1)],
                                 wo[p_][:, QC * eb:QC * (eb + 1)],
                                 start=(p_ == 0), stop=(p_ == 1))
        for eb in range(2):
            nc.vector.tensor_copy(osb[st][:, QC * eb:QC * (eb + 1)],
                                  ops[eb][:])
        nc.sync.dma_start(out.ap()[P * st:P * (st + 1), :], osb[st][:])

    with tc.tile_pool(name="sc_ps", bufs=2, space="PSUM") as sc_ps, \
         tc.tile_pool(name="at_ps", bufs=2, space="PSUM") as at_ps, \
         tc.tile_pool(name="op_ps", bufs=2, space="PSUM") as op_ps, \
         tc.tile_pool(name="e_sb", bufs=3) as e_pool:
        for qc in range(NQC):
            for t in range(2):
                hA, hB = 2 * t, 2 * t + 1
                atA = at_ps.tile([HD + 1, QC], F32, name=f"at{qc}_{hA}",
                                 tag="at")
                atB = at_ps.tile([HD + 1, QC], F32, name=f"at{qc}_{hB}",
                                 tag="at")
                pend = []
                for kk in range(NKK):
                    sc = sc_ps.tile([P, 2 * QC], F32, name=f"sc{qc}_{t}_{kk}",
                                    tag="sc")
                    # two K=64 matmuls in opposite PE row groups -> the two
                    # heads' score tiles stream concurrently
                    nc.tensor.matmul(sc[:, 0:QC],
                                     kt[t][0:HD, P * kk:P * (kk + 1)],
                                     qp[t][0:HD, QC * qc:QC * (qc + 1)],
                                     start=True, stop=True)
                    nc.tensor.matmul(sc[:, QC:2 * QC],
                                     kt[t][HD:P, P * kk:P * (kk + 1)],
                                     qp[t][HD:P, QC * qc:QC * (qc + 1)],
                                     start=True, stop=True)
                    e = e_pool.tile([P, 2 * QC], BF16, name=f"e{qc}_{t}_{kk}",
                                    tag="e")
                    nc.scalar.activation(e[:], sc[:], AF.Exp)
                    if dbg and qc == 0 and t == 0 and kk == 0:
                        nc.sync.dma_start(dbg["dbg_e"].ap(), e[:])
                    # PV one group behind so the PE never waits on the exp
                    if pend:
                        pkk, pe = pend.pop()
                        nc.tensor.matmul(atA[:], vt[pkk][:, 65 * hA:65 * hA + 65],
                                         pe[:, 0:QC],
                                         start=(pkk == 0), stop=(pkk == NKK - 1))
                        nc.tensor.matmul(atB[:], vt[pkk][:, 65 * hB:65 * hB + 65],
                                         pe[:, QC:2 * QC],
                                         start=(pkk == 0), stop=(pkk == NKK - 1))
                    pend.append((kk, e))
                    if kk % 4 == 3 and op_tasks:
                        run_op(op_tasks.pop(0))
                pkk, pe = pend.pop()
                nc.tensor.matmul(atA[:], vt[pkk][:, 65 * hA:65 * hA + 65],
                                 pe[:, 0:QC],
                                 start=(pkk == 0), stop=(pkk == NKK - 1))
                nc.tensor.matmul(atB[:], vt[pkk][:, 65 * hB:65 * hB + 65],
                                 pe[:, QC:2 * QC],
                                 start=(pkk == 0), stop=(pkk == NKK - 1))
                # normalize: attn_sb = at[0:64] / at[64]  (Z from ones col)
                for par, at in ((0, atA), (1, atB)):
                    atsb = small_pool.tile([HD + 1, QC], F32,
                                           name=f"atsb{qc}_{t}_{par}",
                                           tag="atsb")
                    nc.vector.tensor_copy(atsb[:], at[:])
                    zr = small_pool.tile([1, QC], F32,
                                         name=f"zr{qc}_{t}_{par}", tag="zr")
                    nc.vector.tensor_copy(zr[:], atsb[HD:HD + 1, :])
                    rz = small_pool.tile([1, QC], F32,
                                         name=f"rz{qc}_{t}_{par}", tag="rz")
                    nc.vector.reciprocal_approx_fast(rz[:], zr[:])
                    rzb = small_pool.tile([HD, QC], F32,
                                          name=f"rzb{qc}_{t}_{par}", tag="rzb")
                    nc.gpsimd.partition_broadcast(rzb[:], rz[:])
                    nc.vector.tensor_mul(
                        attn_sb[t][HD * par:HD * (par + 1),
                                   QC * qc:QC * (qc + 1)],
                        atsb[0:HD, :], rzb[:])
                    if dbg and qc == 0 and t == 0 and par == 0:
                        nc.sync.dma_start(dbg["dbg_atsb"].ap(), atsb[:])
                        nc.sync.dma_start(dbg["dbg_rz"].ap(), rz[:])
                        nc.sync.dma_start(dbg["dbg_rzb"].ap(), rzb[:])
            # output projection for this query block, deferred one block
            op_tasks.extend(range(4 * qc, 4 * qc + 4))
            if qc == NQC - 1:
                while op_tasks:
                    run_op(op_tasks.pop(0))
        if dbg:
            nc.sync.dma_start(dbg["dbg_asb0"].ap(), attn_sb[0][:])

    ctx.close()


_CACHE = {}


def _get_program():
    if "nc" not in _CACHE:
        _CACHE["nc"] = build_program()
    return _CACHE["nc"]


def prep_inputs(input_tensor, qkv_weight, qkv_bias, out_weight, out_bias):
    """Host-side shard + transpose + cast. Returns in_maps for 8 cores."""
    x = np.asarray(input_tensor, np.float32)
    wqkv = np.asarray(qkv_weight, np.float32).copy()
    bqkv = np.asarray(qkv_bias, np.float32).copy()
    wout = np.asarray(out_weight, np.float32)
    scale = 1.0 / np.sqrt(np.float32(HD))
    wqkv[:D] *= scale
    bqkv[:D] *= scale
    bf = ml_dtypes.bfloat16
    woutT = np.ascontiguousarray(wout.T)
    xTb = [np.ascontiguousarray(x[b].T).astype(bf) for b in range(B)]
    in_maps = []
    for c in range(N_CORES):
        b, g = c // G, c % G
        lo = FL * g
        wqT = np.ascontiguousarray(wqkv[lo:lo + FL, :].T).astype(bf)
        wkT = np.ascontiguousarray(wqkv[D + lo:D + lo + FL, :].T).astype(bf)
        wvT = np.ascontiguousarray(
            wqkv[2 * D + lo:2 * D + lo + FL, :].T).astype(bf)
        woTg = np.ascontiguousarray(woutT[lo:lo + FL, :]).astype(bf)
        bq = bqkv[lo:lo + FL].reshape(2, P).T
        bk = bqkv[D + lo:D + lo + FL].reshape(2, P).T
        bqk = np.ascontiguousarray(np.concatenate([bq, bk], 1)).astype(bf)
        in_maps.append({"xT": xTb[b], "wqT": wqT, "wkT": wkT, "wvT": wvT,
                       "woT": woTg, "bqk": bqk})
    return in_maps


def assemble(outs, qkv_bias, out_weight, out_bias):
    """Sum the per-core partials and add the (V-bias-folded) output bias."""
    bqkv = np.asarray(qkv_bias, np.float32)
    wout = np.asarray(out_weight, np.float32)
    bout_eff = np.asarray(out_bias, np.float32) + wout @ bqkv[2 * D:]
    full = np.empty((B, S, D), np.float32)
    for b in range(B):
        acc = bout_eff[None, :].astype(np.float32).repeat(S, 0)
        for g in range(G):
            acc += np.asarray(outs[b * G + g], np.float32)
        full[b] = acc
    return full


def kernel(input_tensor, qkv_weight, qkv_bias, out_weight, out_bias,
           **run_kwargs):
    nc = _get_program()
    in_maps = prep_inputs(input_tensor, qkv_weight, qkv_bias, out_weight,
                          out_bias)
    res = run_bass_kernel_spmd(nc, in_maps, core_ids=list(range(N_CORES)),
                               **run_kwargs)
    full = assemble([res.results[c]["out"] for c in range(N_CORES)],
                    qkv_bias, out_weight, out_bias)
    if run_kwargs:
        kernel.last_results = res
    return full


# revision 16
# speedup vs baseline: 1.6483x; 1.0090x over previous
"""Multi-head attention (B=2, S=2048, D=1024, H=16) on 8 Trainium2 NeuronCores.

Sharding: core c handles (batch b=c//4, head-group g=c%4 of 4 heads) for ALL
2048 queries — head/tensor parallel instead of the old query-parallel split.
 - Q/K/V projections only cover the core's 256 features (4x less PE work than
   replicating K/V per batch; no collectives needed).
 - Attention (4 heads x 2048 queries x 2048 keys):
   scores^T = K_h^T-pair @ Q_h^T as K=64-contraction matmuls in alternating
   PE row groups (two heads run concurrently in the array),
   exp on ACT at FD=1024, attnT = [V_h|1]^T @ E with 65-col stationaries
   (ones column gives the softmax denominator Z in psum row 64).
 - Normalize uses the fast approximate reciprocal custom DVE op.
 - Output projection contracts only the local 256 features -> each core emits
   a PARTIAL output [2048, 1024] bf16; the host sums the 4 partials per batch
   and adds the (V-bias-folded) output bias.
"""

import numpy as np
import ml_dtypes

import concourse.bass as bass
import concourse.mybir as mybir
import concourse.tile as tile
from concourse import bacc
from concourse.bass_utils import run_bass_kernel_spmd

BF16 = mybir.dt.bfloat16
F32 = mybir.dt.float32
AF = mybir.ActivationFunctionType

B, S, D = 2, 2048, 1024
H, HD = 16, 64
N_CORES = 8
G = 4              # head-groups per batch (cores per batch)
HL = H // G        # heads per core (4)
FL = HL * HD       # local projected features (256)
P = 128
DCH = D // P       # 8 contraction chunks
NKK = S // P       # 16 key chunks
QC = 512           # query block
NQC = S // QC      # 4
VW = HL * (HD + 1) + HD  # packed [V|1] width + 64 pad so 65h+65 slices stay
                         # inside one dense region (pad cols memset to 0)


DEBUG_DUMP = False


def build_program():
    nc = bacc.Bacc("TRN2", target_bir_lowering=False, debug=False,
                   num_devices=N_CORES)

    xT = nc.dram_tensor("xT", [D, S], BF16, kind="ExternalInput")
    wqT = nc.dram_tensor("wqT", [D, FL], BF16, kind="ExternalInput")
    wkT = nc.dram_tensor("wkT", [D, FL], BF16, kind="ExternalInput")
    wvT = nc.dram_tensor("wvT", [D, FL], BF16, kind="ExternalInput")
    woT = nc.dram_tensor("woT", [FL, D], BF16, kind="ExternalInput")
    bqk = nc.dram_tensor("bqk", [P, 4], BF16, kind="ExternalInput")
    out = nc.dram_tensor("out", [S, D], BF16, kind="ExternalOutput")
    dbg = {}
    if DEBUG_DUMP:
        for nm, shape, dt in (
                ("dbg_kt0", [P, S], BF16), ("dbg_qp0", [P, S], BF16),
                ("dbg_vt0", [P, VW], BF16), ("dbg_e", [P, 2 * QC], BF16),
                ("dbg_atsb", [HD + 1, QC], F32), ("dbg_rz", [1, QC], F32),
                ("dbg_rzb", [HD, QC], F32), ("dbg_asb0", [P, S], BF16)):
            dbg[nm] = nc.dram_tensor(nm, shape, dt, kind="ExternalOutput")

    with tile.TileContext(nc) as tc:
        _build(nc, tc, xT, wqT, wkT, wvT, woT, bqk, out, dbg)
    nc.compile()
    return nc


def _build(nc, tc, xT, wqT, wkT, wvT, woT, bqk, out, dbg=()):
    from contextlib import ExitStack

    ctx = ExitStack()
    consts = ctx.enter_context(tc.tile_pool(name="consts", bufs=1))
    bqk_sb = consts.tile([P, 4], BF16, name="bqk_sb")
    nc.sync.dma_start(bqk_sb[:], bqk.ap())

    # ---- resident input tiles ----
    xt_pool = ctx.enter_context(tc.tile_pool(name="xt", bufs=1))
    xt = []
    for i in range(DCH):
        t = xt_pool.tile([P, S], BF16, name=f"xt{i}")
        for ch in range(4):
            nc.sync.dma_start(t[:, QC * ch:QC * (ch + 1)],
                              xT.ap()[P * i:P * (i + 1),
                                      QC * ch:QC * (ch + 1)])
        xt.append(t)

    # ---- weights ----
    w_pool = ctx.enter_context(tc.tile_pool(name="w", bufs=1))
    wk, wq, wv = [], [], []
    for nm, dram, lst in (("wk", wkT, wk), ("wq", wqT, wq), ("wv", wvT, wv)):
        for d in range(DCH):
            t = w_pool.tile([P, FL], BF16, name=f"{nm}{d}")
            nc.gpsimd.dma_start(t[:], dram.ap()[P * d:P * (d + 1), :])
            lst.append(t)
    wo = []
    for p_ in range(2):
        t = w_pool.tile([P, D], BF16, name=f"wo{p_}")
        nc.gpsimd.dma_start(t[:], woT.ap()[P * p_:P * (p_ + 1), :])
        wo.append(t)

    # ---- persistent compute tiles ----
    kv_pool = ctx.enter_context(tc.tile_pool(name="kv", bufs=1))
    kt = [kv_pool.tile([P, S], BF16, name=f"kt{t}") for t in range(2)]
    qp = [kv_pool.tile([P, S], BF16, name=f"qp{t}") for t in range(2)]
    vt = [kv_pool.tile([P, VW], BF16, name=f"vt{g}") for g in range(NKK)]
    for g in range(NKK):
        v3 = vt[g][:, 0:HL * (HD + 1)].rearrange("p (h c) -> p h c", c=HD + 1)
        nc.vector.memset(v3[:, :, HD:HD + 1], 1.0)
        nc.vector.memset(vt[g][:, HL * (HD + 1):VW], 0.0)
    attn_sb = [kv_pool.tile([P, S], BF16, name=f"asb{t}") for t in range(2)]
    osb = [kv_pool.tile([P, D], BF16, name=f"osb{st}")
           for st in range(S // P)]

    # ---- K/Q projections in [128, 1024] psum units (4 in flight) ----
    HS = S // 2
    with tc.tile_pool(name="projkq_ps", bufs=4, space="PSUM") as ps_pool:
        for t in range(2):
            for sh in range(2):
                ps = ps_pool.tile([P, HS], F32, name=f"psk{t}_{sh}",
                                  tag="proj")
                for d in range(DCH):
                    for sch in range(2):
                        co = HS * sh + QC * sch
                        nc.tensor.matmul(ps[:, QC * sch:QC * (sch + 1)],
                                         wk[d][:, P * t:P * (t + 1)],
                                         xt[d][:, co:co + QC],
                                         start=(d == 0), stop=(d == DCH - 1))
                nc.scalar.activation(kt[t][:, HS * sh:HS * (sh + 1)], ps[:],
                                     AF.Identity, bias=bqk_sb[:, 2 + t:3 + t])
            if dbg and t == 0:
                nc.sync.dma_start(dbg["dbg_kt0"].ap(), kt[0][:])
        for t in range(2):
            for sh in range(2):
                ps = ps_pool.tile([P, HS], F32, name=f"psq{t}_{sh}",
                                  tag="proj")
                for d in range(DCH):
                    for sch in range(2):
                        co = HS * sh + QC * sch
                        nc.tensor.matmul(ps[:, QC * sch:QC * (sch + 1)],
                                         wq[d][:, P * t:P * (t + 1)],
                                         xt[d][:, co:co + QC],
                                         start=(d == 0), stop=(d == DCH - 1))
                nc.scalar.activation(qp[t][0:HD, HS * sh:HS * (sh + 1)],
                                     ps[0:HD, :], AF.Identity,
                                     bias=bqk_sb[0:HD, t:t + 1])
                nc.scalar.activation(qp[t][HD:P, HS * sh:HS * (sh + 1)],
                                     ps[HD:P, :], AF.Identity,
                                     bias=bqk_sb[HD:P, t:t + 1])
            if dbg and t == 0:
                nc.sync.dma_start(dbg["dbg_qp0"].ap(), qp[0][:])

    # ---- V projection: natural V[s, e], packed [V|1] layout ----
    with tc.tile_pool(name="projv_ps", bufs=3, space="PSUM") as ps_pool:
        for pr in range(NKK // 2):
            ps = ps_pool.tile([P, 2 * FL], F32, name=f"psv{pr}", tag="proj")
            for half in range(2):
                st = 2 * pr + half
                for d in range(DCH):
                    nc.tensor.matmul(ps[:, FL * half:FL * (half + 1)],
                                     xt[d][:, P * st:P * (st + 1)],
                                     wv[d][:],
                                     start=(d == 0), stop=(d == DCH - 1))
            for half in range(2):
                st = 2 * pr + half
                v3 = vt[st][:, 0:HL * (HD + 1)].rearrange(
                    "p (h c) -> p h c", c=HD + 1)
                nc.vector.tensor_copy(
                    v3[:, :, 0:HD],
                    ps[:, FL * half:FL * (half + 1)].rearrange(
                        "p (h dd) -> p h dd", dd=HD))
                if dbg and st == 0:
                    nc.sync.dma_start(dbg["dbg_vt0"].ap(), vt[0][:])

    # ---- attention + interleaved output projection ----
    small_pool = ctx.enter_context(tc.tile_pool(name="small", bufs=4))
    op_tasks = []

    def run_op(st):
        # partial out[s-chunk, :] = sum_p attn_sb[p][:, chunk]^T @ wo[p]
        ops = [op_ps.tile([P, QC], F32, name=f"op{st}_{eb}", tag="op")
               for eb in range(2)]
        for p_ in range(2):
            for eb in range(2):
                nc.tensor.matmul(ops[eb][:],
                                 attn_sb[p_][:, P * st:P * (st + 1)],
                                 wo[p_][:, QC * eb:QC * (eb + 1)],
                                 start=(p_ == 0), stop=(p_ == 1))
        for eb in range(2):
            nc.vector.tensor_copy(osb[st][:, QC * eb:QC * (eb + 1)],
                                  ops[eb][:])
        nc.sync.dma_start(out.ap()[P * st:P * (st + 1), :], osb[st][:])

    with tc.tile_pool(name="sc_ps", bufs=2, space="PSUM") as sc_ps, \
         tc.tile_pool(name="at_ps", bufs=2, space="PSUM") as at_ps, \
         tc.tile_pool(name="op_ps", bufs=2, space="PSUM") as op_ps, \
         tc.tile_pool(name="e_sb", bufs=3) as e_pool:
        for qc in range(NQC):
            for t in range(2):
                hA, hB = 2 * t, 2 * t + 1
                atA = at_ps.tile([HD + 1, QC], F32, name=f"at{qc}_{hA}",
                                 tag="at")
                atB = at_ps.tile([HD + 1, QC], F32, name=f"at{qc}_{hB}",
                                 tag="at")
                pend = []
                for kk in range(NKK):
                    sc = sc_ps.tile([P, 2 * QC], F32, name=f"sc{qc}_{t}_{kk}",
                                    tag="sc")
                    # two K=64 matmuls in opposite PE row groups -> the two
                    # heads' score tiles stream concurrently
                    nc.tensor.matmul(sc[:, 0:QC],
                                     kt[t][0:HD, P * kk:P * (kk + 1)],
                                     qp[t][0:HD, QC * qc:QC * (qc + 1)],
                                     start=True, stop=True)
                    nc.tensor.matmul(sc[:, QC:2 * QC],
                                     kt[t][HD:P, P * kk:P * (kk + 1)],
                                     qp[t][HD:P, QC * qc:QC * (qc + 1)],
                                     start=True, stop=True)
                    e = e_pool.tile([P, 2 * QC], BF16, name=f"e{qc}_{t}_{kk}",
                                    tag="e")
                    nc.scalar.activation(e[:], sc[:], AF.Exp)
                    if dbg and qc == 0 and t == 0 and kk == 0:
                        nc.sync.dma_start(dbg["dbg_e"].ap(), e[:])
                    # PV one group behind so the PE never waits on the exp
                    if pend:
                        pkk, pe = pend.pop()
                        nc.tensor.matmul(atA[:], vt[pkk][:, 65 * hA:65 * hA + 65],
                                         pe[:, 0:QC],
                                         start=(pkk == 0), stop=(pkk == NKK - 1))
                        nc.tensor.matmul(atB[:], vt[pkk][:, 65 * hB:65 * hB + 65],
                                         pe[:, QC:2 * QC],
                                         start=(pkk == 0), stop=(pkk == NKK - 1))
                    pend.append((kk, e))
                    if kk % 4 == 3 and op_tasks:
                        run_op(op_tasks.pop(0))
                pkk, pe = pend.pop()
                nc.tensor.matmul(atA[:], vt[pkk][:, 65 * hA:65 * hA + 65],
                                 pe[:, 0:QC],
                                 start=(pkk == 0), stop=(pkk == NKK - 1))
                nc.tensor.matmul(atB[:], vt[pkk][:, 65 * hB:65 * hB + 65],
                                 pe[:, QC:2 * QC],
                                 start=(pkk == 0), stop=(pkk == NKK - 1))
                # normalize: attn_sb = at[0:64] / at[64]  (Z from ones col)
                for par, at in ((0, atA), (1, atB)):
                    atsb = small_pool.tile([HD + 1, QC], F32,
                                           name=f"atsb{qc}_{t}_{par}",
                                           tag="atsb")
                    nc.vector.tensor_copy(atsb[:], at[:])
                    zr = small_pool.tile([1, QC], F32,
                                         name=f"zr{qc}_{t}_{par}", tag="zr")
                    nc.vector.tensor_copy(zr[:], atsb[HD:HD + 1, :])
                    rz = small_pool.tile([1, QC], F32,
                                         name=f"rz{qc}_{t}_{par}", tag="rz")
                    nc.vector.reciprocal_approx_fast(rz[:], zr[:])
                    rzb = small_pool.tile([HD, QC], F32,
                                          name=f"rzb{qc}_{t}_{par}", tag="rzb")
                    nc.gpsimd.partition_broadcast(rzb[:], rz[:])
                    nc.vector.tensor_mul(
                        attn_sb[t][HD * par:HD * (par + 1),
                                   QC * qc:QC * (qc + 1)],
                        atsb[0:HD, :], rzb[:])
                    if dbg and qc == 0 and t == 0 and par == 0:
                        nc.sync.dma_start(dbg["dbg_atsb"].ap(), atsb[:])
                        nc.sync.dma_start(dbg["dbg_rz"].ap(), rz[:])
                        nc.sync.dma_start(dbg["dbg_rzb"].ap(), rzb[:])
            # output projection for this query block, deferred one block
            op_tasks.extend(range(4 * qc, 4 * qc + 4))
            if qc == NQC - 1:
                while op_tasks:
                    run_op(op_tasks.pop(0))
        if dbg:
            nc.sync.dma_start(dbg["dbg_asb0"].ap(), attn_sb[0][:])

    ctx.close()


_CACHE = {}


def _get_program():
    if "nc" not in _CACHE:
        _CACHE["nc"] = build_program()
    return _CACHE["nc"]


def prep_inputs(input_tensor, qkv_weight, qkv_bias, out_weight, out_bias):
    """Host-side shard + transpose + cast. Returns in_maps for 8 cores."""
    x = np.asarray(input_tensor, np.float32)
    wqkv = np.asarray(qkv_weight, np.float32).copy()
    bqkv = np.asarray(qkv_bias, np.float32).copy()
    wout = np.asarray(out_weight, np.float32)
    scale = 1.0 / np.sqrt(np.float32(HD))
    wqkv[:D] *= scale
    bqkv[:D] *= scale
    bf = ml_dtypes.bfloat16
    woutT = np.ascontiguousarray(wout.T)
    xTb = [np.ascontiguousarray(x[b].T).astype(bf) for b in range(B)]
    in_maps = []
    for c in range(N_CORES):
        b, g = c // G, c % G
        lo = FL * g
        wqT = np.ascontiguousarray(wqkv[lo:lo + FL, :].T).astype(bf)
        wkT = np.ascontiguousarray(wqkv[D + lo:D + lo + FL, :].T).astype(bf)
        wvT = np.ascontiguousarray(
            wqkv[2 * D + lo:2 * D + lo + FL, :].T).astype(bf)
        woTg = np.ascontiguousarray(woutT[lo:lo + FL, :]).astype(bf)
        bq = bqkv[lo:lo + FL].reshape(2, P).T
        bk = bqkv[D + lo:D + lo + FL].reshape(2, P).T
        bqk = np.ascontiguousarray(np.concatenate([bq, bk], 1)).astype(bf)
        in_maps.append({"xT": xTb[b], "wqT": wqT, "wkT": wkT, "wvT": wvT,
                       "woT": woTg, "bqk": bqk})
    return in_maps


def assemble(outs, qkv_bias, out_weight, out_bias):
    """Sum the per-core partials and add the (V-bias-folded) output bias."""
    bqkv = np.asarray(qkv_bias, np.float32)
    wout = np.asarray(out_weight, np.float32)
    bout_eff = np.asarray(out_bias, np.float32) + wout @ bqkv[2 * D:]
    full = np.empty((B, S, D), np.float32)
    for b in range(B):
        acc = bout_eff[None, :].astype(np.float32).repeat(S, 0)
        for g in range(G):
            acc += np.asarray(outs[b * G + g], np.float32)
        full[b] = acc
    return full


def kernel(input_tensor, qkv_weight, qkv_bias, out_weight, out_bias,
           **run_kwargs):
    nc = _get_program()
    in_maps = prep_inputs(input_tensor, qkv_weight, qkv_bias, out_weight,
                          out_bias)
    res = run_bass_kernel_spmd(nc, in_maps, core_ids=list(range(N_CORES)),
                               **run_kwargs)
    full = assemble([res.results[c]["out"] for c in range(N_CORES)],
                    qkv_bias, out_weight, out_bias)
    if run_kwargs:
        kernel.last_results = res
    return full


# revision 17
# speedup vs baseline: 1.7073x; 1.0358x over previous
"""Multi-head attention (B=2, S=2048, D=1024, H=16) on 8 Trainium2 NeuronCores.

Sharding: core c handles (batch b=c//4, head-group g=c%4 of 4 heads) for ALL
2048 queries — head/tensor parallel instead of the old query-parallel split.
 - Q/K/V projections only cover the core's 256 features (4x less PE work than
   replicating K/V per batch; no collectives needed).
 - Attention (4 heads x 2048 queries x 2048 keys):
   scores^T = K_h^T-pair @ Q_h^T as K=64-contraction matmuls in alternating
   PE row groups (two heads run concurrently in the array),
   exp on ACT at FD=1024, attnT = [V_h|1]^T @ E with 65-col stationaries
   (ones column gives the softmax denominator Z in psum row 64).
 - Normalize uses the fast approximate reciprocal custom DVE op.
 - Output projection contracts only the local 256 features -> each core emits
   a PARTIAL output [2048, 1024] bf16; the host sums the 4 partials per batch
   and adds the (V-bias-folded) output bias.
"""

import numpy as np
import ml_dtypes

import concourse.bass as bass
import concourse.mybir as mybir
import concourse.tile as tile
from concourse import bacc
from concourse.bass_utils import run_bass_kernel_spmd

BF16 = mybir.dt.bfloat16
F32 = mybir.dt.float32
AF = mybir.ActivationFunctionType

B, S, D = 2, 2048, 1024
H, HD = 16, 64
N_CORES = 8
G = 4              # head-groups per batch (cores per batch)
HL = H // G        # heads per core (4)
FL = HL * HD       # local projected features (256)
P = 128
DCH = D // P       # 8 contraction chunks
NKK = S // P       # 16 key chunks
QC = 512           # query block
NQC = S // QC      # 4
VW = HL * (HD + 1) + HD  # packed [V|1] width + 64 pad so 65h+65 slices stay
                         # inside one dense region (pad cols memset to 0)


DEBUG_DUMP = False


def build_program():
    nc = bacc.Bacc("TRN2", target_bir_lowering=False, debug=False,
                   num_devices=N_CORES)

    xT = nc.dram_tensor("xT", [D, S], BF16, kind="ExternalInput")
    wqT = nc.dram_tensor("wqT", [D, FL], BF16, kind="ExternalInput")
    wkT = nc.dram_tensor("wkT", [D, FL], BF16, kind="ExternalInput")
    wvT = nc.dram_tensor("wvT", [D, FL], BF16, kind="ExternalInput")
    woT = nc.dram_tensor("woT", [FL, D], BF16, kind="ExternalInput")
    bqk = nc.dram_tensor("bqk", [P, 4], BF16, kind="ExternalInput")
    out = nc.dram_tensor("out", [S, D], BF16, kind="ExternalOutput")
    dbg = {}
    if DEBUG_DUMP:
        for nm, shape, dt in (
                ("dbg_kt0", [P, S], BF16), ("dbg_qp0", [P, S], BF16),
                ("dbg_vt0", [P, VW], BF16), ("dbg_e", [P, 2 * QC], BF16),
                ("dbg_atsb", [HD + 1, QC], F32), ("dbg_rz", [1, QC], F32),
                ("dbg_rzb", [HD, QC], F32), ("dbg_asb0", [P, S], BF16)):
            dbg[nm] = nc.dram_tensor(nm, shape, dt, kind="ExternalOutput")

    with tile.TileContext(nc) as tc:
        _build(nc, tc, xT, wqT, wkT, wvT, woT, bqk, out, dbg)
    nc.compile()
    return nc


def _build(nc, tc, xT, wqT, wkT, wvT, woT, bqk, out, dbg=()):
    from contextlib import ExitStack

    ctx = ExitStack()
    consts = ctx.enter_context(tc.tile_pool(name="consts", bufs=1))
    bqk_sb = consts.tile([P, 4], BF16, name="bqk_sb")
    nc.sync.dma_start(bqk_sb[:], bqk.ap())

    # ---- resident input tiles: first halves on sync, rest on gpsimd ----
    xt_pool = ctx.enter_context(tc.tile_pool(name="xt", bufs=1))
    xt = [xt_pool.tile([P, S], BF16, name=f"xt{i}") for i in range(DCH)]
    for ch in (0, 1):
        for i in range(DCH):
            nc.sync.dma_start(xt[i][:, QC * ch:QC * (ch + 1)],
                              xT.ap()[P * i:P * (i + 1),
                                      QC * ch:QC * (ch + 1)])

    # ---- weights (gpsimd queue, in consumption order), then xt tails ----
    w_pool = ctx.enter_context(tc.tile_pool(name="w", bufs=1))
    wk, wq, wv = [], [], []
    for nm, dram, lst in (("wk", wkT, wk), ("wq", wqT, wq), ("wv", wvT, wv)):
        for d in range(DCH):
            t = w_pool.tile([P, FL], BF16, name=f"{nm}{d}")
            nc.gpsimd.dma_start(t[:], dram.ap()[P * d:P * (d + 1), :])
            lst.append(t)
    for ch in (2, 3):
        for i in range(DCH):
            nc.gpsimd.dma_start(xt[i][:, QC * ch:QC * (ch + 1)],
                                xT.ap()[P * i:P * (i + 1),
                                        QC * ch:QC * (ch + 1)])
    wo = []
    for p_ in range(2):
        t = w_pool.tile([P, D], BF16, name=f"wo{p_}")
        nc.gpsimd.dma_start(t[:], woT.ap()[P * p_:P * (p_ + 1), :])
        wo.append(t)

    # ---- persistent compute tiles ----
    kv_pool = ctx.enter_context(tc.tile_pool(name="kv", bufs=1))
    kt = [kv_pool.tile([P, S], BF16, name=f"kt{t}") for t in range(2)]
    qp = [kv_pool.tile([P, S], BF16, name=f"qp{t}") for t in range(2)]
    vt = [kv_pool.tile([P, VW], BF16, name=f"vt{g}") for g in range(NKK)]
    for g in range(NKK):
        v3 = vt[g][:, 0:HL * (HD + 1)].rearrange("p (h c) -> p h c", c=HD + 1)
        nc.vector.memset(v3[:, :, HD:HD + 1], 1.0)
        nc.vector.memset(vt[g][:, HL * (HD + 1):VW], 0.0)
    attn_sb = [kv_pool.tile([P, S], BF16, name=f"asb{t}") for t in range(2)]
    osb = [kv_pool.tile([P, D], BF16, name=f"osb{st}")
           for st in range(S // P)]

    # ---- one flat instruction stream: projections, attention, output ----
    # PSUM budget (8 banks): sc 2x[128,1024] = 4, at 2x[65,512] = 2,
    # op/vproj 2x[128,512] = 2. K/Q projection units borrow sc tiles;
    # V projection borrows op tiles, so every phase weaves into the stream.
    HS = S // 2
    small_pool = ctx.enter_context(tc.tile_pool(name="small", bufs=8))

    with tc.tile_pool(name="sc_ps", bufs=2, space="PSUM") as sc_ps, \
         tc.tile_pool(name="at_ps", bufs=2, space="PSUM") as at_ps, \
         tc.tile_pool(name="op_ps", bufs=2, space="PSUM") as op_ps, \
         tc.tile_pool(name="e_sb", bufs=6) as e_pool:

        def proj_unit(which, t, sh):
            ps = sc_ps.tile([P, HS], F32, name=f"ps{which}{t}_{sh}", tag="sc")
            w = wk if which == "k" else wq
            for d in range(DCH):
                for sch in range(2):
                    co = HS * sh + QC * sch
                    nc.tensor.matmul(ps[:, QC * sch:QC * (sch + 1)],
                                     w[d][:, P * t:P * (t + 1)],
                                     xt[d][:, co:co + QC],
                                     start=(d == 0), stop=(d == DCH - 1))
            if which == "k":
                nc.scalar.activation(kt[t][:, HS * sh:HS * (sh + 1)], ps[:],
                                     AF.Identity, bias=bqk_sb[:, 2 + t:3 + t])
                if dbg and t == 0 and sh == 1:
                    nc.sync.dma_start(dbg["dbg_kt0"].ap(), kt[0][:])
            else:
                nc.scalar.activation(qp[t][0:HD, HS * sh:HS * (sh + 1)],
                                     ps[0:HD, :], AF.Identity,
                                     bias=bqk_sb[0:HD, t:t + 1])
                nc.scalar.activation(qp[t][HD:P, HS * sh:HS * (sh + 1)],
                                     ps[HD:P, :], AF.Identity,
                                     bias=bqk_sb[HD:P, t:t + 1])
                if dbg and t == 0 and sh == 1:
                    nc.sync.dma_start(dbg["dbg_qp0"].ap(), qp[0][:])

        def vproj_pair(pr):
            ps = op_ps.tile([P, 2 * FL], F32, name=f"psv{pr}", tag="op")
            for half in range(2):
                st = 2 * pr + half
                for d in range(DCH):
                    nc.tensor.matmul(ps[:, FL * half:FL * (half + 1)],
                                     xt[d][:, P * st:P * (st + 1)], wv[d][:],
                                     start=(d == 0), stop=(d == DCH - 1))
            for half in range(2):
                st = 2 * pr + half
                v3 = vt[st][:, 0:HL * (HD + 1)].rearrange(
                    "p (h c) -> p h c", c=HD + 1)
                nc.vector.tensor_copy(
                    v3[:, :, 0:HD],
                    ps[:, FL * half:FL * (half + 1)].rearrange(
                        "p (h dd) -> p h dd", dd=HD))
            if dbg and pr == 0:
                nc.sync.dma_start(dbg["dbg_vt0"].ap(), vt[0][:])

        def run_op(st):
            ops = [op_ps.tile([P, QC], F32, name=f"op{st}_{eb}", tag="op")
                   for eb in range(2)]
            for p_ in range(2):
                for eb in range(2):
                    nc.tensor.matmul(ops[eb][:],
                                     attn_sb[p_][:, P * st:P * (st + 1)],
                                     wo[p_][:, QC * eb:QC * (eb + 1)],
                                     start=(p_ == 0), stop=(p_ == 1))
            for eb in range(2):
                nc.vector.tensor_copy(osb[st][:, QC * eb:QC * (eb + 1)],
                                      ops[eb][:])
            nc.sync.dma_start(out.ap()[P * st:P * (st + 1), :], osb[st][:])

        def normalize(qc, t, par, at):
            atsb = small_pool.tile([HD + 1, QC], F32,
                                   name=f"atsb{qc}_{t}_{par}", tag="atsb")
            nc.vector.tensor_copy(atsb[:], at[:])
            zr = small_pool.tile([1, QC], F32, name=f"zr{qc}_{t}_{par}",
                                 tag="zr")
            nc.vector.tensor_copy(zr[:], atsb[HD:HD + 1, :])
            rz = small_pool.tile([1, QC], F32, name=f"rz{qc}_{t}_{par}",
                                 tag="rz")
            nc.vector.reciprocal_approx_fast(rz[:], zr[:])
            rzb = small_pool.tile([HD, QC], F32, name=f"rzb{qc}_{t}_{par}",
                                  tag="rzb")
            nc.gpsimd.partition_broadcast(rzb[:], rz[:])
            nc.vector.tensor_mul(
                attn_sb[t][HD * par:HD * (par + 1), QC * qc:QC * (qc + 1)],
                atsb[0:HD, :], rzb[:])
            if dbg and qc == 0 and t == 0 and par == 0:
                nc.sync.dma_start(dbg["dbg_atsb"].ap(), atsb[:])
                nc.sync.dma_start(dbg["dbg_rz"].ap(), rz[:])
                nc.sync.dma_start(dbg["dbg_rzb"].ap(), rzb[:])

        units = [(qc, t, kk, par) for qc in range(NQC) for t in range(2)
                 for kk in range(NKK) for par in range(2)]
        # interjected work, keyed by unit index (deadline-driven):
        # kt[1]/qp[1] before unit 32, all vt pairs before their PV drains,
        # second-half q tiles before unit 128.
        interject = {
            0: [("k", 0, 1)], 2: [("v", 1)], 6: [("v", 2)],
            10: [("k", 1, 0)], 14: [("v", 3)], 18: [("k", 1, 1)],
            22: [("v", 4)], 26: [("q", 1, 0)], 30: [("v", 5)],
            34: [("v", 6)], 38: [("v", 7)], 48: [("q", 0, 1)],
            56: [("q", 1, 1)],
        }
        pend = []            # (qc, t, kk, par, group, e_tile, col_off)
        at_tiles = {}
        vpair_unit = {0: -100}
        op_queue = []
        group_idx = 0
        sc_cur, cur = None, []

        def drain_one(u, force=False):
            if not pend:
                return False
            qc, t, kk, par, g, et, off = pend[0]
            if not force:
                if g >= group_idx:
                    return False
                if vpair_unit.get(kk // 2, 10 ** 9) > u - 3:
                    return False
            pend.pop(0)
            key = (qc, t, par)
            if key not in at_tiles:
                at_tiles[key] = at_ps.tile([HD + 1, QC], F32,
                                           name=f"at{qc}_{t}_{par}", tag="at")
            h = 2 * t + par
            nc.tensor.matmul(at_tiles[key][:], vt[kk][:, 65 * h:65 * h + 65],
                             et[:, off:off + QC],
                             start=(kk == 0), stop=(kk == NKK - 1))
            if kk == NKK - 1:
                normalize(qc, t, par, at_tiles.pop(key))
                if t == 1 and par == 1:
                    op_queue.extend(range(4 * qc, 4 * qc + 4))
            return True

        # preamble: the minimum needed before the first score matmul
        proj_unit("k", 0, 0)
        proj_unit("q", 0, 0)
        vproj_pair(0)

        for u, (qc, t, kk, par) in enumerate(units):
            for ij in interject.get(u, []):
                if ij[0] == "v":
                    vproj_pair(ij[1])
                    vpair_unit[ij[1]] = u
                else:
                    proj_unit(*ij)
            if sc_cur is None:
                sc_cur = sc_ps.tile([P, 2 * QC], F32, name=f"sc{u}", tag="sc")
                cur = []
            nc.tensor.matmul(sc_cur[:, QC * len(cur):QC * (len(cur) + 1)],
                             kt[t][HD * par:HD * (par + 1),
                                   P * kk:P * (kk + 1)],
                             qp[t][HD * par:HD * (par + 1),
                                   QC * qc:QC * (qc + 1)],
                             start=True, stop=True)
            cur.append((qc, t, kk, par))
            if len(cur) == 2:
                e = e_pool.tile([P, 2 * QC], BF16, name=f"e{u}", tag="e")
                nc.scalar.activation(e[:], sc_cur[:], AF.Exp)
                if dbg and u == 1:
                    nc.sync.dma_start(dbg["dbg_e"].ap(), e[:])
                for j, cu in enumerate(cur):
                    pend.append((*cu, group_idx, e, QC * j))
                group_idx += 1
                sc_cur = None
            drained = 0
            while drained < 3 and drain_one(u):
                drained += 1
            if u % 8 == 5 and op_queue:
                run_op(op_queue.pop(0))
        while pend:
            drain_one(10 ** 9, force=True)
        while op_queue:
            run_op(op_queue.pop(0))
        if dbg:
            nc.sync.dma_start(dbg["dbg_asb0"].ap(), attn_sb[0][:])

    ctx.close()


_CACHE = {}


def _get_program():
    if "nc" not in _CACHE:
        _CACHE["nc"] = build_program()
    return _CACHE["nc"]


def prep_inputs(input_tensor, qkv_weight, qkv_bias, out_weight, out_bias):
    """Host-side shard + transpose + cast. Returns in_maps for 8 cores."""
    x = np.asarray(input_tensor, np.float32)
    wqkv = np.asarray(qkv_weight, np.float32).copy()
    bqkv = np.asarray(qkv_bias, np.float32).copy()
    wout = np.asarray(out_weight, np.float32)
    scale = 1.0 / np.sqrt(np.float32(HD))
    wqkv[:D] *= scale
    bqkv[:D] *= scale
    bf = ml_dtypes.bfloat16
    woutT = np.ascontiguousarray(wout.T)
    xTb = [np.ascontiguousarray(x[b].T).astype(bf) for b in range(B)]
    in_maps = []
    for c in range(N_CORES):
        b, g = c // G, c % G
        lo = FL * g
        wqT = np.ascontiguousarray(wqkv[lo:lo + FL, :].T).astype(bf)
        wkT = np.ascontiguousarray(wqkv[D + lo:D + lo + FL, :].T).astype(bf)
        wvT = np.ascontiguousarray(
            wqkv[2 * D + lo:2 * D + lo + FL, :].T).astype(bf)
        woTg = np.ascontiguousarray(woutT[lo:lo + FL, :]).astype(bf)
        bq = bqkv[lo:lo + FL].reshape(2, P).T
        bk = bqkv[D + lo:D + lo + FL].reshape(2, P).T
        bqk = np.ascontiguousarray(np.concatenate([bq, bk], 1)).astype(bf)
        in_maps.append({"xT": xTb[b], "wqT": wqT, "wkT": wkT, "wvT": wvT,
                       "woT": woTg, "bqk": bqk})
    return in_maps


def assemble(outs, qkv_bias, out_weight, out_bias):
    """Sum the per-core partials and add the (V-bias-folded) output bias."""
    bqkv = np.asarray(qkv_bias, np.float32)
    wout = np.asarray(out_weight, np.float32)
    bout_eff = np.asarray(out_bias, np.float32) + wout @ bqkv[2 * D:]
    full = np.empty((B, S, D), np.float32)
    for b in range(B):
        acc = bout_eff[None, :].astype(np.float32).repeat(S, 0)
        for g in range(G):
            acc += np.asarray(outs[b * G + g], np.float32)
        full[b] = acc
    return full


def kernel(input_tensor, qkv_weight, qkv_bias, out_weight, out_bias,
           **run_kwargs):
    nc = _get_program()
    in_maps = prep_inputs(input_tensor, qkv_weight, qkv_bias, out_weight,
                          out_bias)
    res = run_bass_kernel_spmd(nc, in_maps, core_ids=list(range(N_CORES)),
                               **run_kwargs)
    full = assemble([res.results[c]["out"] for c in range(N_CORES)],
                    qkv_bias, out_weight, out_bias)
    if run_kwargs:
        kernel.last_results = res
    return full


# revision 19
# speedup vs baseline: 1.7125x; 1.0031x over previous
"""Multi-head attention (B=2, S=2048, D=1024, H=16) on 8 Trainium2 NeuronCores.

Sharding: core c handles (batch b=c//4, head-group g=c%4 of 4 heads) for ALL
2048 queries — head/tensor parallel instead of the old query-parallel split.
 - Q/K/V projections only cover the core's 256 features (4x less PE work than
   replicating K/V per batch; no collectives needed).
 - Attention (4 heads x 2048 queries x 2048 keys):
   scores^T = K_h^T-pair @ Q_h^T as K=64-contraction matmuls in alternating
   PE row groups (two heads run concurrently in the array),
   exp on ACT at FD=1024, attnT = [V_h|1]^T @ E with 65-col stationaries
   (ones column gives the softmax denominator Z in psum row 64).
 - Normalize uses the fast approximate reciprocal custom DVE op.
 - Output projection contracts only the local 256 features -> each core emits
   a PARTIAL output [2048, 1024] bf16; the host sums the 4 partials per batch
   and adds the (V-bias-folded) output bias.
"""

import numpy as np
import ml_dtypes

import concourse.bass as bass
import concourse.mybir as mybir
import concourse.tile as tile
from concourse import bacc
from concourse.bass_utils import run_bass_kernel_spmd

BF16 = mybir.dt.bfloat16
F32 = mybir.dt.float32
AF = mybir.ActivationFunctionType

B, S, D = 2, 2048, 1024
H, HD = 16, 64
N_CORES = 8
G = 4              # head-groups per batch (cores per batch)
HL = H // G        # heads per core (4)
FL = HL * HD       # local projected features (256)
P = 128
DCH = D // P       # 8 contraction chunks
NKK = S // P       # 16 key chunks
QC = 512           # query block
NQC = S // QC      # 4
VW = HL * (HD + 1) + HD  # packed [V|1] width + 64 pad so 65h+65 slices stay
                         # inside one dense region (pad cols memset to 0)


DEBUG_DUMP = False


def build_program():
    nc = bacc.Bacc("TRN2", target_bir_lowering=False, debug=False,
                   num_devices=N_CORES)

    xT = nc.dram_tensor("xT", [D, S], BF16, kind="ExternalInput")
    wqT = nc.dram_tensor("wqT", [D, FL], BF16, kind="ExternalInput")
    wkT = nc.dram_tensor("wkT", [D, FL], BF16, kind="ExternalInput")
    wvT = nc.dram_tensor("wvT", [D, FL], BF16, kind="ExternalInput")
    woT = nc.dram_tensor("woT", [FL, D], BF16, kind="ExternalInput")
    bqk = nc.dram_tensor("bqk", [P, 4], BF16, kind="ExternalInput")
    out = nc.dram_tensor("out", [S, D], BF16, kind="ExternalOutput")
    dbg = {}
    if DEBUG_DUMP:
        for nm, shape, dt in (
                ("dbg_kt0", [P, S], BF16), ("dbg_qp0", [P, S], BF16),
                ("dbg_vt0", [P, VW], BF16), ("dbg_e", [P, 2 * QC], BF16),
                ("dbg_atsb", [HD + 1, QC], F32), ("dbg_rz", [1, QC], F32),
                ("dbg_rzb", [HD, QC], F32), ("dbg_asb0", [P, S], BF16)):
            dbg[nm] = nc.dram_tensor(nm, shape, dt, kind="ExternalOutput")

    with tile.TileContext(nc) as tc:
        _build(nc, tc, xT, wqT, wkT, wvT, woT, bqk, out, dbg)
    nc.compile()
    return nc


def _build(nc, tc, xT, wqT, wkT, wvT, woT, bqk, out, dbg=()):
    from contextlib import ExitStack

    ctx = ExitStack()
    consts = ctx.enter_context(tc.tile_pool(name="consts", bufs=1))
    bqk_sb = consts.tile([P, 4], BF16, name="bqk_sb")
    nc.sync.dma_start(bqk_sb[:], bqk.ap())

    # ---- resident input tiles: first halves on sync, rest on gpsimd ----
    xt_pool = ctx.enter_context(tc.tile_pool(name="xt", bufs=1))
    xt = [xt_pool.tile([P, S], BF16, name=f"xt{i}") for i in range(DCH)]
    for ch in (0, 1):
        for i in range(DCH):
            nc.sync.dma_start(xt[i][:, QC * ch:QC * (ch + 1)],
                              xT.ap()[P * i:P * (i + 1),
                                      QC * ch:QC * (ch + 1)])

    # ---- weights (gpsimd queue, in consumption order), then xt tails ----
    w_pool = ctx.enter_context(tc.tile_pool(name="w", bufs=1))
    wk, wq, wv = [], [], []
    for nm, dram, lst in (("wk", wkT, wk), ("wq", wqT, wq), ("wv", wvT, wv)):
        for d in range(DCH):
            t = w_pool.tile([P, FL], BF16, name=f"{nm}{d}")
            nc.gpsimd.dma_start(t[:], dram.ap()[P * d:P * (d + 1), :])
            lst.append(t)
    for ch in (2, 3):
        for i in range(DCH):
            nc.gpsimd.dma_start(xt[i][:, QC * ch:QC * (ch + 1)],
                                xT.ap()[P * i:P * (i + 1),
                                        QC * ch:QC * (ch + 1)])
    wo = []
    for p_ in range(2):
        t = w_pool.tile([P, D], BF16, name=f"wo{p_}")
        nc.gpsimd.dma_start(t[:], woT.ap()[P * p_:P * (p_ + 1), :])
        wo.append(t)

    # ---- persistent compute tiles ----
    kv_pool = ctx.enter_context(tc.tile_pool(name="kv", bufs=1))
    kt = [kv_pool.tile([P, S], BF16, name=f"kt{t}") for t in range(2)]
    qz = [kv_pool.tile([P, S], BF16, name=f"qz{h}") for h in range(HL)]
    for h in range(4):
        off = HD * ((h + 1) % 2)
        nc.vector.memset(qz[h][off:off + HD, :], 0.0)
    vt = [kv_pool.tile([P, VW], BF16, name=f"vt{g}") for g in range(NKK)]
    for g in range(NKK):
        v3 = vt[g][:, 0:HL * (HD + 1)].rearrange("p (h c) -> p h c", c=HD + 1)
        nc.vector.memset(v3[:, :, HD:HD + 1], 1.0)
        nc.vector.memset(vt[g][:, HL * (HD + 1):VW], 0.0)
    attn_sb = [kv_pool.tile([P, S], BF16, name=f"asb{t}") for t in range(2)]
    osb = [kv_pool.tile([P, D], BF16, name=f"osb{st}")
           for st in range(S // P)]

    # ---- one flat instruction stream: projections, attention, output ----
    # PSUM budget (8 banks): sc 2x[128,1024] = 4, at 2x[65,512] = 2,
    # op/vproj 2x[128,512] = 2. K/Q projection units borrow sc tiles;
    # V projection borrows op tiles, so every phase weaves into the stream.
    HS = S // 2
    small_pool = ctx.enter_context(tc.tile_pool(name="small", bufs=8))

    with tc.tile_pool(name="sc_ps", bufs=2, space="PSUM") as sc_ps, \
         tc.tile_pool(name="at_ps", bufs=2, space="PSUM") as at_ps, \
         tc.tile_pool(name="op_ps", bufs=2, space="PSUM") as op_ps, \
         tc.tile_pool(name="e_sb", bufs=6) as e_pool:

        def proj_unit(which, t, sh):
            ps = sc_ps.tile([P, HS], F32, name=f"ps{which}{t}_{sh}", tag="sc")
            w = wk if which == "k" else wq
            for d in range(DCH):
                for sch in range(2):
                    co = HS * sh + QC * sch
                    nc.tensor.matmul(ps[:, QC * sch:QC * (sch + 1)],
                                     w[d][:, P * t:P * (t + 1)],
                                     xt[d][:, co:co + QC],
                                     start=(d == 0), stop=(d == DCH - 1))
            if which == "k":
                nc.scalar.activation(kt[t][:, HS * sh:HS * (sh + 1)], ps[:],
                                     AF.Identity, bias=bqk_sb[:, 2 + t:3 + t])
                if dbg and t == 0 and sh == 1:
                    nc.sync.dma_start(dbg["dbg_kt0"].ap(), kt[0][:])
            else:
                nc.scalar.activation(qz[2 * t][0:HD, HS * sh:HS * (sh + 1)],
                                     ps[0:HD, :], AF.Identity,
                                     bias=bqk_sb[0:HD, t:t + 1])
                nc.scalar.activation(qz[2 * t + 1][HD:P, HS * sh:HS * (sh + 1)],
                                     ps[HD:P, :], AF.Identity,
                                     bias=bqk_sb[HD:P, t:t + 1])

        def vproj_pair(pr):
            ps = op_ps.tile([P, 2 * FL], F32, name=f"psv{pr}", tag="op")
            for half in range(2):
                st = 2 * pr + half
                for d in range(DCH):
                    nc.tensor.matmul(ps[:, FL * half:FL * (half + 1)],
                                     xt[d][:, P * st:P * (st + 1)], wv[d][:],
                                     start=(d == 0), stop=(d == DCH - 1))
            for half in range(2):
                st = 2 * pr + half
                v3 = vt[st][:, 0:HL * (HD + 1)].rearrange(
                    "p (h c) -> p h c", c=HD + 1)
                nc.vector.tensor_copy(
                    v3[:, :, 0:HD],
                    ps[:, FL * half:FL * (half + 1)].rearrange(
                        "p (h dd) -> p h dd", dd=HD))
            if dbg and pr == 0:
                nc.sync.dma_start(dbg["dbg_vt0"].ap(), vt[0][:])

        def run_op(st):
            ops = [op_ps.tile([P, QC], F32, name=f"op{st}_{eb}", tag="op")
                   for eb in range(2)]
            for p_ in range(2):
                for eb in range(2):
                    nc.tensor.matmul(ops[eb][:],
                                     attn_sb[p_][:, P * st:P * (st + 1)],
                                     wo[p_][:, QC * eb:QC * (eb + 1)],
                                     start=(p_ == 0), stop=(p_ == 1))
            for eb in range(2):
                nc.vector.tensor_copy(osb[st][:, QC * eb:QC * (eb + 1)],
                                      ops[eb][:])
            nc.sync.dma_start(out.ap()[P * st:P * (st + 1), :], osb[st][:])

        def normalize(qc, t, par, at):
            atsb = small_pool.tile([HD + 1, QC], F32,
                                   name=f"atsb{qc}_{t}_{par}", tag="atsb")
            nc.vector.tensor_copy(atsb[:], at[:])
            zr = small_pool.tile([1, QC], F32, name=f"zr{qc}_{t}_{par}",
                                 tag="zr")
            nc.vector.tensor_copy(zr[:], atsb[HD:HD + 1, :])
            rz = small_pool.tile([1, QC], F32, name=f"rz{qc}_{t}_{par}",
                                 tag="rz")
            nc.vector.reciprocal_approx_fast(rz[:], zr[:])
            rzb = small_pool.tile([HD, QC], F32, name=f"rzb{qc}_{t}_{par}",
                                  tag="rzb")
            nc.gpsimd.partition_broadcast(rzb[:], rz[:])
            nc.vector.tensor_mul(
                attn_sb[t][HD * par:HD * (par + 1), QC * qc:QC * (qc + 1)],
                atsb[0:HD, :], rzb[:])
            if dbg and qc == 0 and t == 0 and par == 0:
                nc.sync.dma_start(dbg["dbg_atsb"].ap(), atsb[:])
                nc.sync.dma_start(dbg["dbg_rz"].ap(), rz[:])
                nc.sync.dma_start(dbg["dbg_rzb"].ap(), rzb[:])

        units = [(qc, t, kk, par) for qc in range(NQC) for t in range(2)
                 for kk in range(NKK) for par in range(2)]
        # interjected work, keyed by unit index (deadline-driven):
        # kt[1]/qp[1] before unit 32, all vt pairs before their PV drains,
        # second-half q tiles before unit 128.
        interject = {
            0: [("k", 0, 1)], 2: [("v", 1)], 6: [("v", 2)],
            10: [("k", 1, 0)], 14: [("v", 3)], 18: [("k", 1, 1)],
            22: [("v", 4)], 26: [("q", 1, 0)], 30: [("v", 5)],
            34: [("v", 6)], 38: [("v", 7)], 48: [("q", 0, 1)],
            56: [("q", 1, 1)],
        }
        pend = []            # (qc, t, kk, par, group, e_tile, col_off)
        at_tiles = {}
        vpair_unit = {0: -100}
        op_queue = []
        group_idx = 0
        sc_cur, cur = None, []

        def drain_one(u, force=False):
            if not pend:
                return False
            qc, t, kk, par, g, et, off = pend[0]
            if not force:
                if g >= group_idx:
                    return False
                if vpair_unit.get(kk // 2, 10 ** 9) > u - 3:
                    return False
            pend.pop(0)
            key = (qc, t, par)
            if key not in at_tiles:
                at_tiles[key] = at_ps.tile([HD + 1, QC], F32,
                                           name=f"at{qc}_{t}_{par}", tag="at")
            h = 2 * t + par
            nc.tensor.matmul(at_tiles[key][:], vt[kk][:, 65 * h:65 * h + 65],
                             et[:, off:off + QC],
                             start=(kk == 0), stop=(kk == NKK - 1))
            if kk == NKK - 1:
                normalize(qc, t, par, at_tiles.pop(key))
                if t == 1 and par == 1:
                    op_queue.extend(range(4 * qc, 4 * qc + 4))
            return True

        # preamble: the minimum needed before the first score matmul
        proj_unit("k", 0, 0)
        proj_unit("q", 0, 0)
        vproj_pair(0)

        for u, (qc, t, kk, par) in enumerate(units):
            for ij in interject.get(u, []):
                if ij[0] == "v":
                    vproj_pair(ij[1])
                    vpair_unit[ij[1]] = u
                else:
                    proj_unit(*ij)
            if sc_cur is None:
                sc_cur = sc_ps.tile([P, 2 * QC], F32, name=f"sc{u}", tag="sc")
                cur = []
            nc.tensor.matmul(sc_cur[:, QC * len(cur):QC * (len(cur) + 1)],
                             kt[t][:, P * kk:P * (kk + 1)],
                             qz[2 * t + par][:, QC * qc:QC * (qc + 1)],
                             start=True, stop=True)
            cur.append((qc, t, kk, par))
            if len(cur) == 2:
                e = e_pool.tile([P, 2 * QC], BF16, name=f"e{u}", tag="e")
                nc.scalar.activation(e[:], sc_cur[:], AF.Exp)
                if dbg and u == 1:
                    nc.sync.dma_start(dbg["dbg_e"].ap(), e[:])
                for j, cu in enumerate(cur):
                    pend.append((*cu, group_idx, e, QC * j))
                group_idx += 1
                sc_cur = None
            drained = 0
            while drained < 3 and drain_one(u):
                drained += 1
            if u % 8 == 5 and op_queue:
                run_op(op_queue.pop(0))
        while pend:
            drain_one(10 ** 9, force=True)
        while op_queue:
            run_op(op_queue.pop(0))
        if dbg:
            nc.sync.dma_start(dbg["dbg_asb0"].ap(), attn_sb[0][:])

    ctx.close()


_CACHE = {}


def _get_program():
    if "nc" not in _CACHE:
        _CACHE["nc"] = build_program()
    return _CACHE["nc"]


def prep_inputs(input_tensor, qkv_weight, qkv_bias, out_weight, out_bias):
    """Host-side shard + transpose + cast. Returns in_maps for 8 cores."""
    x = np.asarray(input_tensor, np.float32)
    wqkv = np.asarray(qkv_weight, np.float32).copy()
    bqkv = np.asarray(qkv_bias, np.float32).copy()
    wout = np.asarray(out_weight, np.float32)
    scale = 1.0 / np.sqrt(np.float32(HD))
    wqkv[:D] *= scale
    bqkv[:D] *= scale
    bf = ml_dtypes.bfloat16
    woutT = np.ascontiguousarray(wout.T)
    xTb = [np.ascontiguousarray(x[b].T).astype(bf) for b in range(B)]
    in_maps = []
    for c in range(N_CORES):
        b, g = c // G, c % G
        lo = FL * g
        wqT = np.ascontiguousarray(wqkv[lo:lo + FL, :].T).astype(bf)
        wkT = np.ascontiguousarray(wqkv[D + lo:D + lo + FL, :].T).astype(bf)
        wvT = np.ascontiguousarray(
            wqkv[2 * D + lo:2 * D + lo + FL, :].T).astype(bf)
        woTg = np.ascontiguousarray(woutT[lo:lo + FL, :]).astype(bf)
        bq = bqkv[lo:lo + FL].reshape(2, P).T
        bk = bqkv[D + lo:D + lo + FL].reshape(2, P).T
        bqk = np.ascontiguousarray(np.concatenate([bq, bk], 1)).astype(bf)
        in_maps.append({"xT": xTb[b], "wqT": wqT, "wkT": wkT, "wvT": wvT,
                       "woT": woTg, "bqk": bqk})
    return in_maps


def assemble(outs, qkv_bias, out_weight, out_bias):
    """Sum the per-core partials and add the (V-bias-folded) output bias."""
    bqkv = np.asarray(qkv_bias, np.float32)
    wout = np.asarray(out_weight, np.float32)
    bout_eff = np.asarray(out_bias, np.float32) + wout @ bqkv[2 * D:]
    full = np.empty((B, S, D), np.float32)
    for b in range(B):
        acc = bout_eff[None, :].astype(np.float32).repeat(S, 0)
        for g in range(G):
            acc += np.asarray(outs[b * G + g], np.float32)
        full[b] = acc
    return full


def kernel(input_tensor, qkv_weight, qkv_bias, out_weight, out_bias,
           **run_kwargs):
    nc = _get_program()
    in_maps = prep_inputs(input_tensor, qkv_weight, qkv_bias, out_weight,
                          out_bias)
    res = run_bass_kernel_spmd(nc, in_maps, core_ids=list(range(N_CORES)),
                               **run_kwargs)
    full = assemble([res.results[c]["out"] for c in range(N_CORES)],
                    qkv_bias, out_weight, out_bias)
    if run_kwargs:
        kernel.last_results = res
    return full


# revision 20
# speedup vs baseline: 1.7402x; 1.0162x over previous
"""Multi-head attention (B=2, S=2048, D=1024, H=16) on 8 Trainium2 NeuronCores.

Sharding: core c handles (batch b=c//4, head-group g=c%4 of 4 heads) for ALL
2048 queries — head/tensor parallel instead of the old query-parallel split.
 - Q/K/V projections only cover the core's 256 features (4x less PE work than
   replicating K/V per batch; no collectives needed).
 - Attention (4 heads x 2048 queries x 2048 keys):
   scores^T = K_h^T-pair @ Q_h^T as K=64-contraction matmuls in alternating
   PE row groups (two heads run concurrently in the array),
   exp on ACT at FD=1024, attnT = [V_h|1]^T @ E with 65-col stationaries
   (ones column gives the softmax denominator Z in psum row 64).
 - Normalize uses the fast approximate reciprocal custom DVE op.
 - Output projection contracts only the local 256 features -> each core emits
   a PARTIAL output [2048, 1024] bf16; the host sums the 4 partials per batch
   and adds the (V-bias-folded) output bias.
"""

import numpy as np
import ml_dtypes

import concourse.bass as bass
import concourse.mybir as mybir
import concourse.tile as tile
from concourse import bacc
from concourse.bass_utils import run_bass_kernel_spmd

BF16 = mybir.dt.bfloat16
F32 = mybir.dt.float32
AF = mybir.ActivationFunctionType

B, S, D = 2, 2048, 1024
H, HD = 16, 64
N_CORES = 8
G = 4              # head-groups per batch (cores per batch)
HL = H // G        # heads per core (4)
FL = HL * HD       # local projected features (256)
P = 128
DCH = D // P       # 8 contraction chunks
NKK = S // P       # 16 key chunks
QC = 512           # query block
NQC = S // QC      # 4
VW = HL * (HD + 1) + HD  # packed [V|1] width + 64 pad so 65h+65 slices stay
                         # inside one dense region (pad cols memset to 0)


DEBUG_DUMP = False


def build_program():
    nc = bacc.Bacc("TRN2", target_bir_lowering=False, debug=False,
                   num_devices=N_CORES)

    xT = nc.dram_tensor("xT", [D, S], BF16, kind="ExternalInput")
    wqT = nc.dram_tensor("wqT", [D, FL], BF16, kind="ExternalInput")
    wkT = nc.dram_tensor("wkT", [D, FL], BF16, kind="ExternalInput")
    wvT = nc.dram_tensor("wvT", [D, FL], BF16, kind="ExternalInput")
    woT = nc.dram_tensor("woT", [FL, D], BF16, kind="ExternalInput")
    bqk = nc.dram_tensor("bqk", [P, 4], BF16, kind="ExternalInput")
    out = nc.dram_tensor("out", [S, D], BF16, kind="ExternalOutput")
    dbg = {}
    if DEBUG_DUMP:
        for nm, shape, dt in (
                ("dbg_kt0", [P, S], BF16), ("dbg_qp0", [P, S], BF16),
                ("dbg_vt0", [P, VW], BF16), ("dbg_e", [P, 2 * QC], BF16),
                ("dbg_atsb", [HD + 1, QC], F32), ("dbg_rz", [1, QC], F32),
                ("dbg_rzb", [HD, QC], F32), ("dbg_asb0", [P, S], BF16)):
            dbg[nm] = nc.dram_tensor(nm, shape, dt, kind="ExternalOutput")

    with tile.TileContext(nc) as tc:
        _build(nc, tc, xT, wqT, wkT, wvT, woT, bqk, out, dbg)
    nc.compile()
    return nc


def _build(nc, tc, xT, wqT, wkT, wvT, woT, bqk, out, dbg=()):
    from contextlib import ExitStack

    ctx = ExitStack()
    consts = ctx.enter_context(tc.tile_pool(name="consts", bufs=1))
    bqk_sb = consts.tile([P, 4], BF16, name="bqk_sb")
    nc.sync.dma_start(bqk_sb[:], bqk.ap())

    # ---- resident input tiles: first halves on sync, rest on gpsimd ----
    xt_pool = ctx.enter_context(tc.tile_pool(name="xt", bufs=1))
    xt = [xt_pool.tile([P, S], BF16, name=f"xt{i}") for i in range(DCH)]
    for ch in (0, 1):
        for i in range(DCH):
            nc.sync.dma_start(xt[i][:, QC * ch:QC * (ch + 1)],
                              xT.ap()[P * i:P * (i + 1),
                                      QC * ch:QC * (ch + 1)])

    # ---- weights (gpsimd queue, in consumption order), then xt tails ----
    w_pool = ctx.enter_context(tc.tile_pool(name="w", bufs=1))
    wk, wq, wv = [], [], []
    for nm, dram, lst in (("wk", wkT, wk), ("wq", wqT, wq), ("wv", wvT, wv)):
        for d in range(DCH):
            t = w_pool.tile([P, FL], BF16, name=f"{nm}{d}")
            nc.gpsimd.dma_start(t[:], dram.ap()[P * d:P * (d + 1), :])
            lst.append(t)
    for ch in (2, 3):
        for i in range(DCH):
            nc.gpsimd.dma_start(xt[i][:, QC * ch:QC * (ch + 1)],
                                xT.ap()[P * i:P * (i + 1),
                                        QC * ch:QC * (ch + 1)])
    wo = []
    for p_ in range(2):
        t = w_pool.tile([P, D], BF16, name=f"wo{p_}")
        nc.gpsimd.dma_start(t[:], woT.ap()[P * p_:P * (p_ + 1), :])
        wo.append(t)

    # ---- persistent compute tiles ----
    kv_pool = ctx.enter_context(tc.tile_pool(name="kv", bufs=1))
    kt = [kv_pool.tile([P, S], BF16, name=f"kt{t}") for t in range(2)]
    qz = [kv_pool.tile([P, S], BF16, name=f"qz{h}") for h in range(HL)]
    for h in range(4):
        off = HD * ((h + 1) % 2)
        nc.vector.memset(qz[h][off:off + HD, :], 0.0)
    vt = [kv_pool.tile([P, VW], BF16, name=f"vt{g}") for g in range(NKK)]
    for g in range(NKK):
        v3 = vt[g][:, 0:HL * (HD + 1)].rearrange("p (h c) -> p h c", c=HD + 1)
        nc.vector.memset(v3[:, :, HD:HD + 1], 1.0)
        nc.vector.memset(vt[g][:, HL * (HD + 1):VW], 0.0)
    attn_sb = [kv_pool.tile([P, S], BF16, name=f"asb{t}") for t in range(2)]
    osb = [kv_pool.tile([P, D], BF16, name=f"osb{st}")
           for st in range(S // P)]

    # ---- one flat instruction stream: projections, attention, output ----
    # PSUM budget (8 banks): sc 2x[128,1024] = 4, at 2x[65,512] = 2,
    # op/vproj 2x[128,512] = 2. K/Q projection units borrow sc tiles;
    # V projection borrows op tiles, so every phase weaves into the stream.
    HS = S // 2
    small_pool = ctx.enter_context(tc.tile_pool(name="small", bufs=8))

    with tc.tile_pool(name="sc_ps", bufs=2, space="PSUM") as sc_ps, \
         tc.tile_pool(name="at_ps", bufs=2, space="PSUM") as at_ps, \
         tc.tile_pool(name="op_ps", bufs=2, space="PSUM") as op_ps, \
         tc.tile_pool(name="e_sb", bufs=6) as e_pool:

        def proj_unit(which, t, sh):
            ps = sc_ps.tile([P, HS], F32, name=f"ps{which}{t}_{sh}", tag="sc")
            w = wk if which == "k" else wq
            for d in range(DCH):
                for sch in range(2):
                    co = HS * sh + QC * sch
                    nc.tensor.matmul(ps[:, QC * sch:QC * (sch + 1)],
                                     w[d][:, P * t:P * (t + 1)],
                                     xt[d][:, co:co + QC],
                                     start=(d == 0), stop=(d == DCH - 1))
            if which == "k":
                nc.scalar.activation(kt[t][:, HS * sh:HS * (sh + 1)], ps[:],
                                     AF.Identity, bias=bqk_sb[:, 2 + t:3 + t])
                if dbg and t == 0 and sh == 1:
                    nc.sync.dma_start(dbg["dbg_kt0"].ap(), kt[0][:])
            else:
                nc.scalar.activation(qz[2 * t][0:HD, HS * sh:HS * (sh + 1)],
                                     ps[0:HD, :], AF.Identity,
                                     bias=bqk_sb[0:HD, t:t + 1])
                nc.scalar.activation(qz[2 * t + 1][HD:P, HS * sh:HS * (sh + 1)],
                                     ps[HD:P, :], AF.Identity,
                                     bias=bqk_sb[HD:P, t:t + 1])

        def vproj_pair(pr):
            ps = op_ps.tile([P, 2 * FL], F32, name=f"psv{pr}", tag="op")
            for half in range(2):
                st = 2 * pr + half
                for d in range(DCH):
                    nc.tensor.matmul(ps[:, FL * half:FL * (half + 1)],
                                     xt[d][:, P * st:P * (st + 1)], wv[d][:],
                                     start=(d == 0), stop=(d == DCH - 1))
            for half in range(2):
                st = 2 * pr + half
                v3 = vt[st][:, 0:HL * (HD + 1)].rearrange(
                    "p (h c) -> p h c", c=HD + 1)
                nc.vector.tensor_copy(
                    v3[:, :, 0:HD],
                    ps[:, FL * half:FL * (half + 1)].rearrange(
                        "p (h dd) -> p h dd", dd=HD))
            if dbg and pr == 0:
                nc.sync.dma_start(dbg["dbg_vt0"].ap(), vt[0][:])

        def run_op(st):
            ops = [op_ps.tile([P, QC], F32, name=f"op{st}_{eb}", tag="op")
                   for eb in range(2)]
            for p_ in range(2):
                for eb in range(2):
                    nc.tensor.matmul(ops[eb][:],
                                     attn_sb[p_][:, P * st:P * (st + 1)],
                                     wo[p_][:, QC * eb:QC * (eb + 1)],
                                     start=(p_ == 0), stop=(p_ == 1))
            for eb in range(2):
                nc.vector.tensor_copy(osb[st][:, QC * eb:QC * (eb + 1)],
                                      ops[eb][:])
            nc.sync.dma_start(out.ap()[P * st:P * (st + 1), :], osb[st][:])

        def normalize(qc, t, par, at):
            atsb = small_pool.tile([HD, QC], F32,
                                   name=f"atsb{qc}_{t}_{par}", tag="atsb")
            nc.vector.tensor_copy(atsb[:], at[0:HD, :])
            zr = small_pool.tile([1, QC], F32, name=f"zr{qc}_{t}_{par}",
                                 tag="zr")
            nc.vector.tensor_copy(zr[:], at[HD:HD + 1, :])
            rz = small_pool.tile([1, QC], F32, name=f"rz{qc}_{t}_{par}",
                                 tag="rz")
            nc.vector.reciprocal_approx_fast(rz[:], zr[:])
            rzb = small_pool.tile([HD, QC], F32, name=f"rzb{qc}_{t}_{par}",
                                  tag="rzb")
            nc.gpsimd.partition_broadcast(rzb[:], rz[:])
            nc.vector.tensor_mul(
                attn_sb[t][HD * par:HD * (par + 1), QC * qc:QC * (qc + 1)],
                atsb[:], rzb[:])
            if dbg and qc == 0 and t == 0 and par == 0:
                nc.sync.dma_start(dbg["dbg_atsb"].ap(), atsb[:])
                nc.sync.dma_start(dbg["dbg_rz"].ap(), rz[:])
                nc.sync.dma_start(dbg["dbg_rzb"].ap(), rzb[:])

        units = [(qc, t, kk, par) for qc in range(NQC) for t in range(2)
                 for kk in range(NKK) for par in range(2)]
        # interjected work, keyed by unit index (deadline-driven):
        # kt[1]/qp[1] before unit 32, all vt pairs before their PV drains,
        # second-half q tiles before unit 128.
        interject = {
            0: [("k", 0, 1)], 2: [("v", 1)], 6: [("v", 2)],
            10: [("k", 1, 0)], 14: [("v", 3)], 18: [("k", 1, 1)],
            22: [("v", 4)], 26: [("q", 1, 0)], 30: [("v", 5)],
            34: [("v", 6)], 38: [("v", 7)], 48: [("q", 0, 1)],
            56: [("q", 1, 1)],
        }
        pend = []            # (qc, t, kk, par, group, e_tile, col_off)
        at_tiles = {}
        vpair_unit = {0: -100}
        op_queue = []
        group_idx = 0
        sc_cur, cur = None, []

        def drain_one(u, force=False):
            if not pend:
                return False
            qc, t, kk, par, g, et, off = pend[0]
            if not force:
                if g >= group_idx - (1 if kk == 0 else 0):
                    return False
                if vpair_unit.get(kk // 2, 10 ** 9) > u - 3:
                    return False
            pend.pop(0)
            key = (qc, t, par)
            if key not in at_tiles:
                at_tiles[key] = at_ps.tile([HD + 1, QC], F32,
                                           name=f"at{qc}_{t}_{par}", tag="at")
            h = 2 * t + par
            nc.tensor.matmul(at_tiles[key][:], vt[kk][:, 65 * h:65 * h + 65],
                             et[:, off:off + QC],
                             start=(kk == 0), stop=(kk == NKK - 1))
            if kk == NKK - 1:
                normalize(qc, t, par, at_tiles.pop(key))
                if t == 1 and par == 1:
                    op_queue.extend((st, u + 10) for st in
                                    range(4 * qc, 4 * qc + 4))
            return True

        def proj_unit_fast(which, t):
            # sch-outer with split casts: the first 512 columns complete as
            # soon as the first DMA'd chunk of every xt tile lands
            ps = sc_ps.tile([P, HS], F32, name=f"psf{which}{t}", tag="sc")
            w = wk if which == "k" else wq
            for sch in range(2):
                for d in range(DCH):
                    nc.tensor.matmul(ps[:, QC * sch:QC * (sch + 1)],
                                     w[d][:, P * t:P * (t + 1)],
                                     xt[d][:, QC * sch:QC * (sch + 1)],
                                     start=(d == 0), stop=(d == DCH - 1))
                if which == "k":
                    nc.scalar.activation(
                        kt[t][:, QC * sch:QC * (sch + 1)],
                        ps[:, QC * sch:QC * (sch + 1)], AF.Identity,
                        bias=bqk_sb[:, 2 + t:3 + t])
                else:
                    nc.scalar.activation(
                        qz[2 * t][0:HD, QC * sch:QC * (sch + 1)],
                        ps[0:HD, QC * sch:QC * (sch + 1)], AF.Identity,
                        bias=bqk_sb[0:HD, t:t + 1])
                    nc.scalar.activation(
                        qz[2 * t + 1][HD:P, QC * sch:QC * (sch + 1)],
                        ps[HD:P, QC * sch:QC * (sch + 1)], AF.Identity,
                        bias=bqk_sb[HD:P, t:t + 1])

        # preamble: the minimum needed before the first score matmul
        proj_unit_fast("k", 0)
        proj_unit_fast("q", 0)
        vproj_pair(0)

        for u, (qc, t, kk, par) in enumerate(units):
            for ij in interject.get(u, []):
                if ij[0] == "v":
                    vproj_pair(ij[1])
                    vpair_unit[ij[1]] = u
                else:
                    proj_unit(*ij)
            if sc_cur is None:
                sc_cur = sc_ps.tile([P, 2 * QC], F32, name=f"sc{u}", tag="sc")
                cur = []
            nc.tensor.matmul(sc_cur[:, QC * len(cur):QC * (len(cur) + 1)],
                             kt[t][:, P * kk:P * (kk + 1)],
                             qz[2 * t + par][:, QC * qc:QC * (qc + 1)],
                             start=True, stop=True)
            cur.append((qc, t, kk, par))
            if len(cur) == 2:
                e = e_pool.tile([P, 2 * QC], BF16, name=f"e{u}", tag="e")
                nc.scalar.activation(e[:], sc_cur[:], AF.Exp)
                if dbg and u == 1:
                    nc.sync.dma_start(dbg["dbg_e"].ap(), e[:])
                for j, cu in enumerate(cur):
                    pend.append((*cu, group_idx, e, QC * j))
                group_idx += 1
                sc_cur = None
            drained = 0
            while drained < 3 and drain_one(u):
                drained += 1
            if u % 8 == 5 and op_queue and op_queue[0][1] <= u:
                run_op(op_queue.pop(0)[0])
        while pend:
            drain_one(10 ** 9, force=True)
        while op_queue:
            run_op(op_queue.pop(0)[0])
        if dbg:
            nc.sync.dma_start(dbg["dbg_asb0"].ap(), attn_sb[0][:])

    ctx.close()


_CACHE = {}


def _get_program():
    if "nc" not in _CACHE:
        _CACHE["nc"] = build_program()
    return _CACHE["nc"]


def prep_inputs(input_tensor, qkv_weight, qkv_bias, out_weight, out_bias):
    """Host-side shard + transpose + cast. Returns in_maps for 8 cores."""
    x = np.asarray(input_tensor, np.float32)
    wqkv = np.asarray(qkv_weight, np.float32).copy()
    bqkv = np.asarray(qkv_bias, np.float32).copy()
    wout = np.asarray(out_weight, np.float32)
    scale = 1.0 / np.sqrt(np.float32(HD))
    wqkv[:D] *= scale
    bqkv[:D] *= scale
    bf = ml_dtypes.bfloat16
    woutT = np.ascontiguousarray(wout.T)
    xTb = [np.ascontiguousarray(x[b].T).astype(bf) for b in range(B)]
    in_maps = []
    for c in range(N_CORES):
        b, g = c // G, c % G
        lo = FL * g
        wqT = np.ascontiguousarray(wqkv[lo:lo + FL, :].T).astype(bf)
        wkT = np.ascontiguousarray(wqkv[D + lo:D + lo + FL, :].T).astype(bf)
        wvT = np.ascontiguousarray(
            wqkv[2 * D + lo:2 * D + lo + FL, :].T).astype(bf)
        woTg = np.ascontiguousarray(woutT[lo:lo + FL, :]).astype(bf)
        bq = bqkv[lo:lo + FL].reshape(2, P).T
        bk = bqkv[D + lo:D + lo + FL].reshape(2, P).T
        bqk = np.ascontiguousarray(np.concatenate([bq, bk], 1)).astype(bf)
        in_maps.append({"xT": xTb[b], "wqT": wqT, "wkT": wkT, "wvT": wvT,
                       "woT": woTg, "bqk": bqk})
    return in_maps


def assemble(outs, qkv_bias, out_weight, out_bias):
    """Sum the per-core partials and add the (V-bias-folded) output bias."""
    bqkv = np.asarray(qkv_bias, np.float32)
    wout = np.asarray(out_weight, np.float32)
    bout_eff = np.asarray(out_bias, np.float32) + wout @ bqkv[2 * D:]
    full = np.empty((B, S, D), np.float32)
    for b in range(B):
        acc = bout_eff[None, :].astype(np.float32).repeat(S, 0)
        for g in range(G):
            acc += np.asarray(outs[b * G + g], np.float32)
        full[b] = acc
    return full


def kernel(input_tensor, qkv_weight, qkv_bias, out_weight, out_bias,
           **run_kwargs):
    nc = _get_program()
    in_maps = prep_inputs(input_tensor, qkv_weight, qkv_bias, out_weight,
                          out_bias)
    res = run_bass_kernel_spmd(nc, in_maps, core_ids=list(range(N_CORES)),
                               **run_kwargs)
    full = assemble([res.results[c]["out"] for c in range(N_CORES)],
                    qkv_bias, out_weight, out_bias)
    if run_kwargs:
        kernel.last_results = res
    return full
